# revision 29
# baseline (speedup 1.0000x reference)
"""Trainium2 Bass kernel for multi-head attention with RoPE (B=2, S=2048,
D=2048, H=16), distributed over 8 NeuronCores with head tensor-parallelism
and an AllToAll to switch to token-parallelism for the output projection.

kernel(**inputs) takes the full unsharded inputs (as produced by the
reference setup_inputs) and returns the full [2, 2048, 2048] f32 output.

Layout strategy: x is pre-transposed/cast to bf16 [D, T] on the host (same
spirit as the host-side weight transposes), so QKV matmuls stream straight
from SBUF xT tiles with no on-device staging. V is produced directly in
natural [t, hd] layout by swapping matmul operands. The output projection
is split into per-head halves so head-0's half overlaps the second
AllToAll.
"""
import numpy as np
import ml_dtypes
import bass_rust
from concourse import bass, bacc, tile, mybir
from concourse.bass_utils import run_bass_kernel_spmd

bf16 = ml_dtypes.bfloat16
BF16 = mybir.dt.bfloat16
F32 = mybir.dt.float32
AF = mybir.ActivationFunctionType
OP = mybir.AluOpType

B, S, D, H = 2, 2048, 2048, 16
HD = 128                 # head dim
NCORES = 8
HL = H // NCORES         # heads per core = 2
EL = HL * HD             # local projection width = 256
T = B * S                # 4096 flattened tokens
NG = 4                   # 1024-token groups in QKV phase
TG = T // NG             # 1024
NKT = S // 128           # 16 key tiles per batch
NQC = S // 512           # 4 query chunks per batch
NDT = D // 128           # 16 contraction tiles
TL = T // NCORES         # 512 tokens per core after AllToAll
SCALE = float(1.0 / np.sqrt(128.0))

_CACHE = {}


def _build():
    nc = bacc.Bacc("TRN2", target_bir_lowering=False, num_devices=NCORES)

    x_t = nc.dram_tensor("x_t", [D, T], BF16, kind="ExternalInput")
    wq_t = nc.dram_tensor("wq_t", [128, NDT * EL], BF16, kind="ExternalInput")
    wk_t = nc.dram_tensor("wk_t", [128, NDT * EL], BF16, kind="ExternalInput")
    wv_t = nc.dram_tensor("wv_t", [128, NDT * EL], BF16, kind="ExternalInput")
    wo_t = nc.dram_tensor("wo_t", [128, NDT * D], BF16, kind="ExternalInput")
    cos_t = nc.dram_tensor("cos_t", [HD, S], BF16, kind="ExternalInput")
    sin_m = nc.dram_tensor("sin_m", [HD, S], BF16, kind="ExternalInput")
    mask_t = nc.dram_tensor("mask_t", [128, B * NKT], F32, kind="ExternalInput")
    out = nc.dram_tensor("out", [TL, D], F32, kind="ExternalOutput")

    ones_dram = nc.inline_tensor(np.ones((128, 128), dtype=bf16), name="ones")

    with tile.TileContext(nc) as tc:
        with (
            tc.tile_pool(name="dram", bufs=1, space="DRAM") as dram,
            tc.tile_pool(name="consts", bufs=1) as consts,
            tc.tile_pool(name="keep", bufs=1) as keep,
        ):
            # AllToAll buffers. Head 0: ONE collective with full TL-wide
            # shards — 1KB contiguous lines halve the SDMA descriptor count
            # (descriptor rate ~8ns/512B caps split buffers at ~60GB/s) and
            # h0 has ~25us of slack anyway. Head 1 is on the critical path:
            # 2-way token split so the second half wire-pipelines under the
            # output projection of the first.
            a2a_in0 = dram.tile([NCORES, HD, TL], BF16,
                                tag="a2a_in0", name="a2a_in0")
            a2a_out0 = dram.tile([NCORES, HD, TL], BF16,
                                 tag="a2a_out0", name="a2a_out0")
            a2a_in1 = [dram.tile([NCORES, HD, TL // 2], BF16,
                                 tag=f"a2a_in1_{hf}", name=f"a2a_in1_{hf}")
                       for hf in range(2)]
            a2a_out1 = [dram.tile([NCORES, HD, TL // 2], BF16,
                                  tag=f"a2a_out1_{hf}", name=f"a2a_out1_{hf}")
                        for hf in range(2)]

            ones_sb = consts.tile([128, 128], BF16, tag="ones", name="ones_sb")
            nc.scalar.dma_start(ones_sb[:], ones_dram[:])
            mask_sb = consts.tile([128, B * NKT], F32, tag="mask", name="mask_sb")
            nc.scalar.dma_start(mask_sb[:], mask_t[:])
            cos_sb = consts.tile([128, S], BF16, tag="cos", name="cos_sb")
            sin_sb = consts.tile([128, S], BF16, tag="sin", name="sin_sb")

            # persistent per-head tensors: qT/kT in [hd, t]; v natural packed
            # per 128-token block as [t=128, (eh, hd)] along the free dim
            qT = [keep.tile([128, T], BF16, tag=f"qT{h}", name=f"qT{h}")
                  for h in range(HL)]
            kT = [keep.tile([128, T], BF16, tag=f"kT{h}", name=f"kT{h}")
                  for h in range(HL)]
            vnat = keep.tile([128, 2 * T], BF16, tag="vnat", name="vnat")

            # ---------- phase A+B: QKV projections + RoPE ----------
            with (
                tc.tile_pool(name="wsb", bufs=1) as wpool,
                tc.tile_pool(name="xt", bufs=32) as xtpool,
                tc.tile_pool(name="rope", bufs=3) as rope,
                tc.tile_pool(name="qkps", bufs=8, space="PSUM") as qkps,
            ):
                wsb = {}
                for nm, wt in (("q", wq_t), ("k", wk_t), ("v", wv_t)):
                    wsb[nm] = wpool.tile([128, NDT * EL], BF16, tag=f"w{nm}",
                                         name=f"w{nm}")

                # throwaway stationary/moving for PE warmups: memset (DVE)
                # instead of a DMA so the PE can start before any HBM
                # traffic lands
                warm_sb = wpool.tile([128, 128], BF16, tag="warm", name="warm")
                nc.vector.memset(warm_sb[:], 0)

                def emit_rope(ps, nm, eh, t0):
                    pos0 = t0 % S
                    dst = qT[eh] if nm == "q" else kT[eh]
                    tmp = rope.tile([128, 512], F32, tag="ropetmp",
                                    name="ropetmp")
                    nc.vector.tensor_tensor(
                        tmp[:], ps[:], cos_sb[:, pos0:pos0 + 512],
                        OP.mult)
                    u = rope.tile([128, 512], F32, tag="ropeu",
                                  name="ropeu")
                    nc.vector.tensor_tensor(
                        u[0:64, :], ps[64:128, :],
                        sin_sb[0:64, pos0:pos0 + 512], OP.mult)
                    nc.vector.tensor_tensor(
                        u[64:128, :], ps[0:64, :],
                        sin_sb[64:128, pos0:pos0 + 512], OP.mult)
                    nc.vector.tensor_tensor(
                        dst[:, t0:t0 + 512], tmp[:], u[:], OP.add)

                for g in range(NG):
                    g0 = g * TG
                    xts = []
                    for dti in range(NDT):
                        xtile = xtpool.tile([128, TG], BF16, tag="xt", name="xt")
                        if g == 0:
                            # interleave wq AND wk eighth-pairs with the
                            # first group's xt stream: the chunk covering
                            # chains (dti, dti+1) lands before xt[dti], and
                            # the per-slot DMA load stays smooth (~0.4MB) so
                            # tile arrivals pace evenly against PE work
                            if dti % 2 == 0:
                                c0 = (dti // 2) * (NDT * EL // 8)
                                c1 = (dti // 2 + 1) * (NDT * EL // 8)
                                nc.sync.dma_start(wsb["q"][:, c0:c1],
                                                  wq_t[:, c0:c1])
                                nc.sync.dma_start(wsb["k"][:, c0:c1],
                                                  wk_t[:, c0:c1])
                        xq = nc.scalar if (g == 0 and dti % 2 == 1) else nc.sync
                        xq.dma_start(
                            xtile[:], x_t[dti * 128:(dti + 1) * 128, g0:g0 + TG])
                        xts.append(xtile)
                        if g == 0 and dti == NDT - 1:
                            # cos/sin (rope tables, first used at the g0 chain
                            # tail ~30us in) and wv (~45us) stay out of the
                            # startup-critical DMA prefix
                            nc.sync.dma_start(cos_sb[:], cos_t[:])
                            nc.sync.dma_start(sin_sb[:], sin_m[:])
                            nc.sync.dma_start(wsb["v"][:], wv_t[:])
                    if g == 0:
                        # group 0 is paced by the xt/w DMA stream: run all 8
                        # chains (q/k x head x half) d-outer through dti 11 so
                        # each arriving xt tile unlocks 8 matmuls (~3.4us of
                        # cold-clock PE work vs ~1.5us DMA spacing); then
                        # finish the chains ONE AT A TIME (dti 12-15 + rope)
                        # so the chain stops stagger and the rope DVE burst
                        # pipelines into the V phase instead of gating it
                        chains = [(nm, eh, half) for nm in ("q", "k")
                                  for eh in range(HL) for half in range(2)]
                        pss = {c: qkps.tile([128, 512], F32, tag="qkps",
                                            name="qkps") for c in chains}
                        # bridge the ~2.8us before the first xt tile lands
                        # (keeps HAM warm); the first real start=True matmul
                        # clears this bank
                        for _ in range(26):
                            nc.tensor.matmul(
                                pss[("q", 0, 0)][:, 0:128],
                                warm_sb[:], warm_sb[:],
                                start=True, stop=True)
                        for dti in range(12):
                            for nm, eh, half in chains:
                                nc.tensor.matmul(
                                    pss[(nm, eh, half)][:],
                                    wsb[nm][:, dti * EL + eh * 128:
                                            dti * EL + (eh + 1) * 128],
                                    xts[dti][:, half * 512:(half + 1) * 512],
                                    start=(dti == 0), stop=False)
                        for nm, eh, half in chains:
                            for dti in range(12, NDT):
                                nc.tensor.matmul(
                                    pss[(nm, eh, half)][:],
                                    wsb[nm][:, dti * EL + eh * 128:
                                            dti * EL + (eh + 1) * 128],
                                    xts[dti][:, half * 512:(half + 1) * 512],
                                    start=False, stop=(dti == NDT - 1))
                            emit_rope(pss[(nm, eh, half)], nm, eh,
                                      g0 + half * 512)
                    else:
                        # steady state: half-major emission keeps each half's
                        # rope (DVE) hidden under the next half's matmuls
                        for half in range(2):
                            t0 = g0 + half * 512
                            for nm, eh in [(nm, eh) for nm in ("q", "k")
                                           for eh in range(HL)]:
                                ps = qkps.tile([128, 512], F32, tag="qkps",
                                               name="qkps")
                                for dti in range(NDT):
                                    nc.tensor.matmul(
                                        ps[:],
                                        wsb[nm][:, dti * EL + eh * 128:
                                                dti * EL + (eh + 1) * 128],
                                        xts[dti][:, half * 512:(half + 1) * 512],
                                        start=(dti == 0), stop=(dti == NDT - 1))
                                emit_rope(ps, nm, eh, t0)
                    for tb in range(TG // 128):
                        t0 = g0 + tb * 128
                        # V psum shares the qkps ring (padded to the same
                        # [128,512] slot) so the QKV phase fits in 8 banks
                        ps = qkps.tile([128, EL], F32, tag="qkps",
                                       padded_shape=[128, 512], name="vps")
                        for dti in range(NDT):
                            nc.tensor.matmul(
                                ps[:],
                                xts[dti][:, tb * 128:(tb + 1) * 128],
                                wsb["v"][:, dti * EL:(dti + 1) * EL],
                                start=(dti == 0), stop=(dti == NDT - 1))
                        nc.vector.tensor_copy(
                            vnat[:, t0 * 2:t0 * 2 + EL], ps[:])

            # ---------- phase C: SDPA per (head, batch, 1024-query block) ----------
            ot_sb = {}
            mm_hold = [None]
            with (
                tc.tile_pool(name="late", bufs=1) as late,
                # small SBUF pools that outlive the SDPA PSUM scope: the
                # FINAL block's denominator/normalize/a2a are emitted inside
                # the oproj phase so the PE never idles at the transition
                tc.tile_pool(name="onorm", bufs=6) as onpool,
                tc.tile_pool(name="rec", bufs=3) as recpool,
            ):
                wo_sb = late.tile([128, NDT * D], BF16, tag="wo", name="wo_sb")
                otpool = late

                def emit_norm_a2a(h, b, qp, get_den, get_ops):
                    for qc2 in range(2):
                        rec = recpool.tile([128, 512], F32, tag="rec",
                                           name="rec")
                        nc.vector.reciprocal_approx_fast(
                            rec[:], get_den(qc2))
                        on = onpool.tile([128, 512], BF16, tag="on",
                                         name="onorm")
                        nc.vector.tensor_tensor(
                            on[:], get_ops(qc2), rec[:], OP.mult)
                        j = b * NQC + qp * 2 + qc2
                        if h == 0:
                            nc.sync.dma_start(a2a_in0[j, :, :], on[:])
                        else:
                            for hf in range(2):
                                nc.sync.dma_start(
                                    a2a_in1[hf][j, :, :],
                                    on[:, hf * 256:(hf + 1) * 256])

                def emit_collective_fetch(h):
                    # fetches ride the sync ring: NOT the scalar ring
                    # (blocks ScalarE's exp stream) and NOT the gpsimd ring
                    # (measured: Pool-queue DMAs wedge the collective train
                    # — op0 went 26us -> 113us)
                    otile = otpool.tile([128, NCORES * TL], BF16,
                                        tag=f"ot{h}", name=f"ot{h}")
                    if h == 0:
                        nc.gpsimd.collective_compute(
                            "AllToAll", OP.bypass,
                            replica_groups=[list(range(NCORES))],
                            ins=[a2a_in0.opt()],
                            outs=[a2a_out0.opt()],
                        )
                        for src in range(NCORES):
                            nc.sync.dma_start(
                                otile[:, src * TL:(src + 1) * TL],
                                a2a_out0[src, :, :])
                    else:
                        for hf in range(2):
                            nc.gpsimd.collective_compute(
                                "AllToAll", OP.bypass,
                                replica_groups=[list(range(NCORES))],
                                ins=[a2a_in1[hf].opt()],
                                outs=[a2a_out1[hf].opt()],
                            )
                        for hf in range(2):
                            for src in range(NCORES):
                                nc.sync.dma_start(
                                    otile[:, src * TL + hf * 256:
                                          src * TL + (hf + 1) * 256],
                                    a2a_out1[hf][src, :, :])
                    ot_sb[h] = otile

                with (
                    tc.tile_pool(name="E", bufs=16) as epool,
                    tc.tile_pool(name="Epair", bufs=4) as eppool,
                    tc.tile_pool(name="Equad", bufs=3) as eqpool,
                    tc.tile_pool(name="sps", bufs=2, space="PSUM") as spool,
                    tc.tile_pool(name="ops", bufs=2, space="PSUM") as opool,
                ):
                    for h in range(HL):
                        for b in range(B):
                            q0 = b * S
                            for qp in range(2):
                                qb = q0 + qp * 1024
                                last_block = (h == HL - 1 and b == B - 1
                                              and qp == 1)
                                ops_ps = [opool.tile([128, 512], F32, tag="ops",
                                                     name="opsum")
                                          for _ in range(2)]
                                E = []
                                pairs = []
                                quads = []
                                octs = []

                                def attn_step(kt):
                                    e_t = E[kt]
                                    vcol = (b * NKT + kt) * EL + h * 128
                                    for qc2 in range(2):
                                        mm_hold[0] = nc.tensor.matmul(
                                            ops_ps[qc2][:],
                                            vnat[:, vcol:vcol + 128],
                                            e_t[:, qc2 * 512:(qc2 + 1) * 512],
                                            start=(kt == 0), stop=(kt == NKT - 1))
                                    if kt % 2 == 1:
                                        ep = eppool.tile([128, 1024], BF16,
                                                         tag="epair", name="epair")
                                        nc.vector.tensor_tensor(
                                            ep[:], E[kt - 1][:], e_t[:], OP.add)
                                        pairs.append(ep)
                                    if kt % 4 == 3:
                                        eq = eqpool.tile([128, 1024], BF16,
                                                         tag="equad", name="equad")
                                        nc.vector.tensor_tensor(
                                            eq[:], pairs[-2][:], pairs[-1][:],
                                            OP.add)
                                        quads.append(eq)
                                    if kt % 8 == 7:
                                        eo8 = eqpool.tile([128, 1024], BF16,
                                                          tag="eoct", name="eoct")
                                        nc.vector.tensor_tensor(
                                            eo8[:], quads[-2][:], quads[-1][:],
                                            OP.add)
                                        octs.append(eo8)

                                for kt in range(NKT):
                                    sp = spool.tile([128, 1024], F32, tag="sps",
                                                    name="spsum")
                                    for qc2 in range(2):
                                        nc.tensor.matmul(
                                            sp[:, qc2 * 512:(qc2 + 1) * 512],
                                            kT[h][:, q0 + kt * 128:
                                                  q0 + (kt + 1) * 128],
                                            qT[h][:, qb + qc2 * 512:
                                                  qb + (qc2 + 1) * 512],
                                            start=True, stop=True)
                                    e_t = epool.tile([128, 1024], BF16, tag="E",
                                                     name="etile")
                                    mcol = b * NKT + kt
                                    nc.scalar.activation(
                                        e_t[:], sp[:], AF.Exp,
                                        bias=mask_sb[:, mcol:mcol + 1],
                                        scale=SCALE)
                                    E.append(e_t)
                                    # lag the PV/denominator consumption FOUR
                                    # score tiles behind the exp producer: the
                                    # 8-matmul PV backlog at the block tail
                                    # then covers the exp->tree->dps->norm
                                    # serial chain, so the next block's PV(0)
                                    # never stalls on the ops-psum release.
                                    # The FINAL block instead uses lag 2: no
                                    # next block exists, and a short tail gets
                                    # the last AllToAll triggered ~3us sooner
                                    # (the trigger is on the critical path).
                                    lag = 2 if last_block else 4
                                    if kt >= lag:
                                        attn_step(kt - lag)
                                for kt in range(NKT - lag, NKT):
                                    attn_step(kt)

                                # full 16-level tree: one ones-matmul pair per
                                # block (not per oct); its psum has its own
                                # 2-bank slot so the scores ring never waits
                                # on the rec/norm chain
                                e16 = eqpool.tile([128, 1024], BF16,
                                                  tag="eoct", name="e16")
                                nc.vector.tensor_tensor(
                                    e16[:], octs[-2][:], octs[-1][:], OP.add)
                                dps_ps = spool.tile([128, 1024], F32,
                                                    tag="dps", bufs=1,
                                                    name="dpsum")
                                for qc2 in range(2):
                                    mm_hold[0] = nc.tensor.matmul(
                                        dps_ps[:, qc2 * 512:(qc2 + 1) * 512],
                                        ones_sb[:],
                                        e16[:, qc2 * 512:(qc2 + 1) * 512],
                                        start=True, stop=True)
                                if last_block:
                                    # free the ops psum early: the norm reads
                                    # an SBUF copy, so the oproj psum pool's
                                    # boundary only waits on the dps recs
                                    opc = [onpool.tile([128, 512], F32,
                                                       tag="opscopy", bufs=2,
                                                       name="opsc")
                                           for _ in range(2)]
                                    for qc2 in range(2):
                                        nc.vector.tensor_copy(
                                            opc[qc2][:], ops_ps[qc2][:])
                                    emit_norm_a2a(
                                        h, b, qp,
                                        lambda qc2: dps_ps[:, qc2 * 512:
                                                           (qc2 + 1) * 512],
                                        lambda qc2: opc[qc2][:])
                                else:
                                    emit_norm_a2a(
                                        h, b, qp,
                                        lambda qc2: dps_ps[:, qc2 * 512:
                                                           (qc2 + 1) * 512],
                                        lambda qc2: ops_ps[qc2][:])
                        emit_collective_fetch(h)
                        if h == 0:
                            # wo is only needed at the output projection; fetch it
                            # during SDPA when HBM is otherwise quiet
                            nc.scalar.dma_start(wo_sb[:], wo_t[:])

                # ---------- phase D: output projection, split per head ----------
                # head-0's half runs while head-1's AllToAll is in flight
                with (
                    tc.tile_pool(name="ysb", bufs=1) as ypool,
                    tc.tile_pool(name="ysum", bufs=4) as ysumpool,
                    tc.tile_pool(name="yps", bufs=2, space="PSUM") as yppool,
                ):
                    # ordering anchor: keep the output-projection chains after
                    # the SDPA tail in the PE stream — the scheduler's cost
                    # model undershoots the AllToAll latency and would
                    # otherwise front-run these and stall the PE
                    anchor = mm_hold[0]
                    for hh in range(HL):
                        if hh == 1:
                            # bridge a possible wait for the second AllToAll
                            # with a few throwaway matmuls (the first real
                            # start=True clears the bank). Anchor them on the
                            # last h0 chain's FINAL matmul so they cannot
                            # interleave into that chain.
                            anchor = last_mm
                            warm_yp = yppool.tile([128, 512], F32, tag="yps",
                                                  name="warmyp")
                            for _ in range(12):
                                wmm = nc.tensor.matmul(
                                    warm_yp[:, 0:128], ones_sb[:], ones_sb[:],
                                    start=True, stop=True)
                                bass_rust.add_dep_helper(
                                    wmm.ins, anchor.ins, sync=False,
                                    reason="keep PE warm across a2a#1 wait")
                                anchor = wmm
                        for tt in range(TL // 128):
                            for eo in range(4):
                                yp = yppool.tile([128, 512], F32, tag="yps",
                                                 name="ypsum")
                                for di in range(NCORES):
                                    d = di * HL + hh
                                    mm = nc.tensor.matmul(
                                        yp[:],
                                        ot_sb[hh][:, di * TL + tt * 128:
                                                  di * TL + (tt + 1) * 128],
                                        wo_sb[:, d * D + eo * 512:
                                              d * D + (eo + 1) * 512],
                                        start=(di == 0), stop=(di == NCORES - 1))
                                    if di == 0:
                                        bass_rust.add_dep_helper(
                                            mm.ins, anchor.ins, sync=False,
                                            reason="order oproj after prior phase")
                                        anchor = mm
                                    last_mm = mm
                                if hh == 0:
                                    y0 = ypool.tile([128, 512], F32,
                                                    tag=f"y0_{tt}_{eo}",
                                                    name=f"y0_{tt}_{eo}")
                                    nc.vector.tensor_copy(y0[:], yp[:])
                                    ot_sb[(0, tt, eo)] = y0
                                else:
                                    ys = ysumpool.tile([128, 512], F32, tag="ysum",
                                                       name="ysum")
                                    nc.vector.tensor_tensor(
                                        ys[:], yp[:], ot_sb[(0, tt, eo)][:], OP.add)
                                    nc.sync.dma_start(
                                        out[tt * 128:(tt + 1) * 128,
                                            eo * 512:(eo + 1) * 512], ys[:])


    nc.compile()
    return nc


def _prep_in_maps(x, cos, sin, attn_mask, wq, wk, wv, wo):
    x_t = np.ascontiguousarray(
        np.asarray(x, np.float32).reshape(T, D).T.astype(bf16))      # [D, T]
    cosT = np.ascontiguousarray(np.asarray(cos[0], np.float32).T)    # [HD, S]
    sinT = np.asarray(sin[0], np.float32).T
    sin_m = np.ascontiguousarray(
        np.concatenate([-sinT[:64], sinT[64:]], axis=0))             # [HD, S]
    mask_t = np.ascontiguousarray(
        np.asarray(attn_mask, np.float32).reshape(B * NKT, 128).T)   # [128, 32]

    def pack(w_sl):
        # [E_out, D] slice -> [128, NDT * E_out] d-tile-major layout
        e_out = w_sl.shape[0]
        return np.ascontiguousarray(
            w_sl.T.reshape(NDT, 128, e_out).transpose(1, 0, 2)
            .reshape(128, NDT * e_out).astype(bf16))

    wo_t = pack(np.asarray(wo, np.float32))
    in_maps = []
    for i in range(NCORES):
        sl = slice(i * EL, (i + 1) * EL)
        in_maps.append({
            "x_t": x_t,
            "wq_t": pack(np.asarray(wq, np.float32)[sl]),
            "wk_t": pack(np.asarray(wk, np.float32)[sl]),
            "wv_t": pack(np.asarray(wv, np.float32)[sl]),
            "wo_t": wo_t,
            "cos_t": cosT.astype(bf16),
            "sin_m": sin_m.astype(bf16),
            "mask_t": mask_t,
        })
    return in_maps


def kernel(x, cos, sin, attn_mask, wq, wk, wv, wo, _trace=False):
    if "nc" not in _CACHE:
        _CACHE["nc"] = _build()
    nc = _CACHE["nc"]
    in_maps = _prep_in_maps(x, cos, sin, attn_mask, wq, wk, wv, wo)
    res = run_bass_kernel_spmd(nc, in_maps, core_ids=list(range(NCORES)),
                               trace=_trace)
    _CACHE["last_result"] = res
    y = np.concatenate([np.asarray(res.results[i]["out"], np.float32)
                        for i in range(NCORES)], axis=0)
    return y.reshape(B, S, D)



# revision 30
# speedup vs baseline: 1.0002x; 1.0002x over previous
"""Trainium2 Bass kernel for multi-head attention with RoPE (B=2, S=2048,
D=2048, H=16), distributed over 8 NeuronCores with head tensor-parallelism
and an AllToAll to switch to token-parallelism for the output projection.

kernel(**inputs) takes the full unsharded inputs (as produced by the
reference setup_inputs) and returns the full [2, 2048, 2048] f32 output.

Layout strategy: x is pre-transposed/cast to bf16 [D, T] on the host (same
spirit as the host-side weight transposes), so QKV matmuls stream straight
from SBUF xT tiles with no on-device staging. V is produced directly in
natural [t, hd] layout by swapping matmul operands. The output projection
is split into per-head halves so head-0's half overlaps the second
AllToAll.
"""
import numpy as np
import ml_dtypes
import bass_rust
from concourse import bass, bacc, tile, mybir
from concourse.bass_utils import run_bass_kernel_spmd

bf16 = ml_dtypes.bfloat16
BF16 = mybir.dt.bfloat16
F32 = mybir.dt.float32
AF = mybir.ActivationFunctionType
OP = mybir.AluOpType

B, S, D, H = 2, 2048, 2048, 16
HD = 128                 # head dim
NCORES = 8
HL = H // NCORES         # heads per core = 2
EL = HL * HD             # local projection width = 256
T = B * S                # 4096 flattened tokens
NG = 4                   # 1024-token groups in QKV phase
TG = T // NG             # 1024
NKT = S // 128           # 16 key tiles per batch
NQC = S // 512           # 4 query chunks per batch
NDT = D // 128           # 16 contraction tiles
TL = T // NCORES         # 512 tokens per core after AllToAll
SCALE = float(1.0 / np.sqrt(128.0))

_CACHE = {}


def _build():
    nc = bacc.Bacc("TRN2", target_bir_lowering=False, num_devices=NCORES)

    x_t = nc.dram_tensor("x_t", [D, T], BF16, kind="ExternalInput")
    wq_t = nc.dram_tensor("wq_t", [128, NDT * EL], BF16, kind="ExternalInput")
    wk_t = nc.dram_tensor("wk_t", [128, NDT * EL], BF16, kind="ExternalInput")
    wv_t = nc.dram_tensor("wv_t", [128, NDT * EL], BF16, kind="ExternalInput")
    wo_t = nc.dram_tensor("wo_t", [128, NDT * D], BF16, kind="ExternalInput")
    cos_t = nc.dram_tensor("cos_t", [HD, S], BF16, kind="ExternalInput")
    sin_m = nc.dram_tensor("sin_m", [HD, S], BF16, kind="ExternalInput")
    mask_t = nc.dram_tensor("mask_t", [128, B * NKT], F32, kind="ExternalInput")
    out = nc.dram_tensor("out", [TL, D], F32, kind="ExternalOutput")

    ones_dram = nc.inline_tensor(np.ones((128, 128), dtype=bf16), name="ones")

    with tile.TileContext(nc) as tc:
        with (
            tc.tile_pool(name="dram", bufs=1, space="DRAM") as dram,
            tc.tile_pool(name="consts", bufs=1) as consts,
            tc.tile_pool(name="keep", bufs=1) as keep,
        ):
            # AllToAll buffers. Head 0: ONE collective with full TL-wide
            # shards — 1KB contiguous lines halve the SDMA descriptor count
            # (descriptor rate ~8ns/512B caps split buffers at ~60GB/s) and
            # h0 has ~25us of slack anyway. Head 1 is on the critical path:
            # 2-way token split so the second half wire-pipelines under the
            # output projection of the first.
            a2a_in0 = dram.tile([NCORES, HD, TL], BF16,
                                tag="a2a_in0", name="a2a_in0")
            a2a_out0 = dram.tile([NCORES, HD, TL], BF16,
                                 tag="a2a_out0", name="a2a_out0")
            bar_in = dram.tile([NCORES, 1, 64], BF16,
                               tag="bar_in", name="bar_in")
            bar_out = dram.tile([NCORES, 1, 64], BF16,
                                tag="bar_out", name="bar_out")
            a2a_in1 = [dram.tile([NCORES, HD, TL // 2], BF16,
                                 tag=f"a2a_in1_{hf}", name=f"a2a_in1_{hf}")
                       for hf in range(2)]
            a2a_out1 = [dram.tile([NCORES, HD, TL // 2], BF16,
                                  tag=f"a2a_out1_{hf}", name=f"a2a_out1_{hf}")
                        for hf in range(2)]

            ones_sb = consts.tile([128, 128], BF16, tag="ones", name="ones_sb")
            nc.scalar.dma_start(ones_sb[:], ones_dram[:])
            mask_sb = consts.tile([128, B * NKT], F32, tag="mask", name="mask_sb")
            nc.scalar.dma_start(mask_sb[:], mask_t[:])
            cos_sb = consts.tile([128, S], BF16, tag="cos", name="cos_sb")
            sin_sb = consts.tile([128, S], BF16, tag="sin", name="sin_sb")

            # persistent per-head tensors: qT/kT in [hd, t]; v natural packed
            # per 128-token block as [t=128, (eh, hd)] along the free dim
            qT = [keep.tile([128, T], BF16, tag=f"qT{h}", name=f"qT{h}")
                  for h in range(HL)]
            kT = [keep.tile([128, T], BF16, tag=f"kT{h}", name=f"kT{h}")
                  for h in range(HL)]
            vnat = keep.tile([128, 2 * T], BF16, tag="vnat", name="vnat")

            # ---------- phase A+B: QKV projections + RoPE ----------
            with (
                tc.tile_pool(name="wsb", bufs=1) as wpool,
                tc.tile_pool(name="xt", bufs=32) as xtpool,
                tc.tile_pool(name="rope", bufs=3) as rope,
                tc.tile_pool(name="qkps", bufs=8, space="PSUM") as qkps,
            ):
                wsb = {}
                for nm, wt in (("q", wq_t), ("k", wk_t), ("v", wv_t)):
                    wsb[nm] = wpool.tile([128, NDT * EL], BF16, tag=f"w{nm}",
                                         name=f"w{nm}")

                # throwaway stationary/moving for PE warmups: memset (DVE)
                # instead of a DMA so the PE can start before any HBM
                # traffic lands
                warm_sb = wpool.tile([128, 128], BF16, tag="warm", name="warm")
                nc.vector.memset(warm_sb[:], 0)

                def emit_rope(ps, nm, eh, t0):
                    pos0 = t0 % S
                    dst = qT[eh] if nm == "q" else kT[eh]
                    tmp = rope.tile([128, 512], F32, tag="ropetmp",
                                    name="ropetmp")
                    nc.vector.tensor_tensor(
                        tmp[:], ps[:], cos_sb[:, pos0:pos0 + 512],
                        OP.mult)
                    u = rope.tile([128, 512], F32, tag="ropeu",
                                  name="ropeu")
                    nc.vector.tensor_tensor(
                        u[0:64, :], ps[64:128, :],
                        sin_sb[0:64, pos0:pos0 + 512], OP.mult)
                    nc.vector.tensor_tensor(
                        u[64:128, :], ps[0:64, :],
                        sin_sb[64:128, pos0:pos0 + 512], OP.mult)
                    nc.vector.tensor_tensor(
                        dst[:, t0:t0 + 512], tmp[:], u[:], OP.add)

                for g in range(NG):
                    g0 = g * TG
                    xts = []
                    for dti in range(NDT):
                        xtile = xtpool.tile([128, TG], BF16, tag="xt", name="xt")
                        if g == 0:
                            # interleave wq AND wk eighth-pairs with the
                            # first group's xt stream: the chunk covering
                            # chains (dti, dti+1) lands before xt[dti], and
                            # the per-slot DMA load stays smooth (~0.4MB) so
                            # tile arrivals pace evenly against PE work
                            if dti % 2 == 0:
                                c0 = (dti // 2) * (NDT * EL // 8)
                                c1 = (dti // 2 + 1) * (NDT * EL // 8)
                                nc.sync.dma_start(wsb["q"][:, c0:c1],
                                                  wq_t[:, c0:c1])
                                nc.sync.dma_start(wsb["k"][:, c0:c1],
                                                  wk_t[:, c0:c1])
                        xq = nc.scalar if (g == 0 and dti % 2 == 1) else nc.sync
                        xq.dma_start(
                            xtile[:], x_t[dti * 128:(dti + 1) * 128, g0:g0 + TG])
                        xts.append(xtile)
                        if g == 0 and dti == NDT - 1:
                            # cos/sin (rope tables, first used at the g0 chain
                            # tail ~30us in) and wv (~45us) stay out of the
                            # startup-critical DMA prefix
                            nc.sync.dma_start(cos_sb[:], cos_t[:])
                            nc.sync.dma_start(sin_sb[:], sin_m[:])
                            nc.sync.dma_start(wsb["v"][:], wv_t[:])
                    if g == 0:
                        # group 0 is paced by the xt/w DMA stream: run all 8
                        # chains (q/k x head x half) d-outer through dti 11 so
                        # each arriving xt tile unlocks 8 matmuls (~3.4us of
                        # cold-clock PE work vs ~1.5us DMA spacing); then
                        # finish the chains ONE AT A TIME (dti 12-15 + rope)
                        # so the chain stops stagger and the rope DVE burst
                        # pipelines into the V phase instead of gating it
                        chains = [(nm, eh, half) for nm in ("q", "k")
                                  for eh in range(HL) for half in range(2)]
                        pss = {c: qkps.tile([128, 512], F32, tag="qkps",
                                            name="qkps") for c in chains}
                        # bridge the ~2.8us before the first xt tile lands
                        # (keeps HAM warm); the first real start=True matmul
                        # clears this bank
                        for _ in range(26):
                            nc.tensor.matmul(
                                pss[("q", 0, 0)][:, 0:128],
                                warm_sb[:], warm_sb[:],
                                start=True, stop=True)
                        for dti in range(12):
                            for nm, eh, half in chains:
                                nc.tensor.matmul(
                                    pss[(nm, eh, half)][:],
                                    wsb[nm][:, dti * EL + eh * 128:
                                            dti * EL + (eh + 1) * 128],
                                    xts[dti][:, half * 512:(half + 1) * 512],
                                    start=(dti == 0), stop=False)
                        for nm, eh, half in chains:
                            for dti in range(12, NDT):
                                nc.tensor.matmul(
                                    pss[(nm, eh, half)][:],
                                    wsb[nm][:, dti * EL + eh * 128:
                                            dti * EL + (eh + 1) * 128],
                                    xts[dti][:, half * 512:(half + 1) * 512],
                                    start=False, stop=(dti == NDT - 1))
                            emit_rope(pss[(nm, eh, half)], nm, eh,
                                      g0 + half * 512)
                    else:
                        # steady state: half-major emission keeps each half's
                        # rope (DVE) hidden under the next half's matmuls
                        for half in range(2):
                            t0 = g0 + half * 512
                            for nm, eh in [(nm, eh) for nm in ("q", "k")
                                           for eh in range(HL)]:
                                ps = qkps.tile([128, 512], F32, tag="qkps",
                                               name="qkps")
                                for dti in range(NDT):
                                    nc.tensor.matmul(
                                        ps[:],
                                        wsb[nm][:, dti * EL + eh * 128:
                                                dti * EL + (eh + 1) * 128],
                                        xts[dti][:, half * 512:(half + 1) * 512],
                                        start=(dti == 0), stop=(dti == NDT - 1))
                                emit_rope(ps, nm, eh, t0)
                    for tb in range(TG // 128):
                        t0 = g0 + tb * 128
                        # V psum shares the qkps ring (padded to the same
                        # [128,512] slot) so the QKV phase fits in 8 banks
                        ps = qkps.tile([128, EL], F32, tag="qkps",
                                       padded_shape=[128, 512], name="vps")
                        for dti in range(NDT):
                            nc.tensor.matmul(
                                ps[:],
                                xts[dti][:, tb * 128:(tb + 1) * 128],
                                wsb["v"][:, dti * EL:(dti + 1) * EL],
                                start=(dti == 0), stop=(dti == NDT - 1))
                        nc.vector.tensor_copy(
                            vnat[:, t0 * 2:t0 * 2 + EL], ps[:])

            # ---------- phase C: SDPA per (head, batch, 1024-query block) ----------
            ot_sb = {}
            mm_hold = [None]
            with (
                tc.tile_pool(name="late", bufs=1) as late,
                # small SBUF pools that outlive the SDPA PSUM scope: the
                # FINAL block's denominator/normalize/a2a are emitted inside
                # the oproj phase so the PE never idles at the transition
                tc.tile_pool(name="onorm", bufs=6) as onpool,
                tc.tile_pool(name="rec", bufs=3) as recpool,
            ):
                wo_sb = late.tile([128, NDT * D], BF16, tag="wo", name="wo_sb")
                otpool = late

                bar_src = [None]

                def emit_norm_a2a(h, b, qp, get_den, get_ops):
                    for qc2 in range(2):
                        rec = recpool.tile([128, 512], F32, tag="rec",
                                           name="rec")
                        nc.vector.reciprocal_approx_fast(
                            rec[:], get_den(qc2))
                        on = onpool.tile([128, 512], BF16, tag="on",
                                         name="onorm")
                        nc.vector.tensor_tensor(
                            on[:], get_ops(qc2), rec[:], OP.mult)
                        bar_src[0] = on
                        j = b * NQC + qp * 2 + qc2
                        if h == 0:
                            nc.sync.dma_start(a2a_in0[j, :, :], on[:])
                        else:
                            for hf in range(2):
                                nc.sync.dma_start(
                                    a2a_in1[hf][j, :, :],
                                    on[:, hf * 256:(hf + 1) * 256])

                def emit_collective_fetch(h):
                    # fetches ride the sync ring: NOT the scalar ring
                    # (blocks ScalarE's exp stream) and NOT the gpsimd ring
                    # (measured: Pool-queue DMAs wedge the collective train
                    # — op0 went 26us -> 113us)
                    otile = otpool.tile([128, NCORES * TL], BF16,
                                        tag=f"ot{h}", name=f"ot{h}")
                    if h == 0:
                        nc.gpsimd.collective_compute(
                            "AllToAll", OP.bypass,
                            replica_groups=[list(range(NCORES))],
                            ins=[a2a_in0.opt()],
                            outs=[a2a_out0.opt()],
                        )
                        for src in range(NCORES):
                            nc.sync.dma_start(
                                otile[:, src * TL:(src + 1) * TL],
                                a2a_out0[src, :, :])
                    else:
                        for hf in range(2):
                            nc.gpsimd.collective_compute(
                                "AllToAll", OP.bypass,
                                replica_groups=[list(range(NCORES))],
                                ins=[a2a_in1[hf].opt()],
                                outs=[a2a_out1[hf].opt()],
                            )
                        for hf in range(2):
                            for src in range(NCORES):
                                nc.sync.dma_start(
                                    otile[:, src * TL + hf * 256:
                                          src * TL + (hf + 1) * 256],
                                    a2a_out1[hf][src, :, :])
                    ot_sb[h] = otile

                with (
                    tc.tile_pool(name="E", bufs=16) as epool,
                    tc.tile_pool(name="Epair", bufs=4) as eppool,
                    tc.tile_pool(name="Equad", bufs=3) as eqpool,
                    tc.tile_pool(name="sps", bufs=3, space="PSUM") as spool,
                    tc.tile_pool(name="ops", bufs=2, space="PSUM") as opool,
                ):
                    for h in range(HL):
                        for b in range(B):
                            q0 = b * S
                            for qp in range(2):
                                qb = q0 + qp * 1024
                                last_block = (h == HL - 1 and b == B - 1
                                              and qp == 1)
                                ops_ps = [opool.tile([128, 512], F32, tag="ops",
                                                     name="opsum")
                                          for _ in range(2)]
                                E = []
                                pairs = []
                                quads = []
                                octs = []

                                def attn_step(kt):
                                    e_t = E[kt]
                                    vcol = (b * NKT + kt) * EL + h * 128
                                    for qc2 in range(2):
                                        mm_hold[0] = nc.tensor.matmul(
                                            ops_ps[qc2][:],
                                            vnat[:, vcol:vcol + 128],
                                            e_t[:, qc2 * 512:(qc2 + 1) * 512],
                                            start=(kt == 0), stop=(kt == NKT - 1))
                                    if kt % 2 == 1:
                                        ep = eppool.tile([128, 1024], BF16,
                                                         tag="epair", name="epair")
                                        nc.vector.tensor_tensor(
                                            ep[:], E[kt - 1][:], e_t[:], OP.add)
                                        pairs.append(ep)
                                    if kt % 4 == 3:
                                        eq = eqpool.tile([128, 1024], BF16,
                                                         tag="equad", name="equad")
                                        nc.vector.tensor_tensor(
                                            eq[:], pairs[-2][:], pairs[-1][:],
                                            OP.add)
                                        quads.append(eq)
                                    if kt % 8 == 7:
                                        eo8 = eqpool.tile([128, 1024], BF16,
                                                          tag="eoct", name="eoct")
                                        nc.vector.tensor_tensor(
                                            eo8[:], quads[-2][:], quads[-1][:],
                                            OP.add)
                                        octs.append(eo8)

                                for kt in range(NKT):
                                    sp = spool.tile([128, 1024], F32, tag="sps",
                                                    name="spsum")
                                    for qc2 in range(2):
                                        nc.tensor.matmul(
                                            sp[:, qc2 * 512:(qc2 + 1) * 512],
                                            kT[h][:, q0 + kt * 128:
                                                  q0 + (kt + 1) * 128],
                                            qT[h][:, qb + qc2 * 512:
                                                  qb + (qc2 + 1) * 512],
                                            start=True, stop=True)
                                    e_t = epool.tile([128, 1024], BF16, tag="E",
                                                     name="etile")
                                    mcol = b * NKT + kt
                                    nc.scalar.activation(
                                        e_t[:], sp[:], AF.Exp,
                                        bias=mask_sb[:, mcol:mcol + 1],
                                        scale=SCALE)
                                    E.append(e_t)
                                    # lag the PV/denominator consumption FOUR
                                    # score tiles behind the exp producer: the
                                    # 8-matmul PV backlog at the block tail
                                    # then covers the exp->tree->dps->norm
                                    # serial chain, so the next block's PV(0)
                                    # never stalls on the ops-psum release.
                                    # The FINAL block instead uses lag 2: no
                                    # next block exists, and a short tail gets
                                    # the last AllToAll triggered ~3us sooner
                                    # (the trigger is on the critical path).
                                    lag = 2 if last_block else 4
                                    if kt >= lag:
                                        attn_step(kt - lag)
                                for kt in range(NKT - lag, NKT):
                                    attn_step(kt)

                                # full 16-level tree: one ones-matmul pair per
                                # block (not per oct); its psum has its own
                                # 2-bank slot so the scores ring never waits
                                # on the rec/norm chain
                                e16 = eqpool.tile([128, 1024], BF16,
                                                  tag="eoct", name="e16")
                                nc.vector.tensor_tensor(
                                    e16[:], octs[-2][:], octs[-1][:], OP.add)
                                # the denominator psum borrows a scores
                                # ring slot: with the lag-4 PV backlog the
                                # next block's sc2' reaches the slot ~7.5us
                                # after sc15, by which time the recs have
                                # read it — so the ring stays 3 deep and the
                                # scores pipeline decouples from ACT
                                dps_ps = spool.tile([128, 1024], F32,
                                                    tag="sps", name="dpsum")
                                for qc2 in range(2):
                                    mm_hold[0] = nc.tensor.matmul(
                                        dps_ps[:, qc2 * 512:(qc2 + 1) * 512],
                                        ones_sb[:],
                                        e16[:, qc2 * 512:(qc2 + 1) * 512],
                                        start=True, stop=True)
                                if last_block:
                                    # free the ops psum early: the norm reads
                                    # an SBUF copy, so the oproj psum pool's
                                    # boundary only waits on the dps recs
                                    opc = [onpool.tile([128, 512], F32,
                                                       tag="opscopy", bufs=2,
                                                       name="opsc")
                                           for _ in range(2)]
                                    for qc2 in range(2):
                                        nc.vector.tensor_copy(
                                            opc[qc2][:], ops_ps[qc2][:])
                                    emit_norm_a2a(
                                        h, b, qp,
                                        lambda qc2: dps_ps[:, qc2 * 512:
                                                           (qc2 + 1) * 512],
                                        lambda qc2: opc[qc2][:])
                                else:
                                    emit_norm_a2a(
                                        h, b, qp,
                                        lambda qc2: dps_ps[:, qc2 * 512:
                                                           (qc2 + 1) * 512],
                                        lambda qc2: ops_ps[qc2][:])
                                if h == 1 and b == 0 and qp == 1:
                                    # sync barrier, gated on this block's
                                    # output so it fires ~mid-SDPA: absorbs
                                    # the inter-core skew (10-25us) while the
                                    # PE keeps crunching, so the h1 AllToAll
                                    # that follows runs at wire rate instead
                                    # of paying the skew on the critical path
                                    last_on = bar_src[0]
                                    for jj in range(NCORES):
                                        nc.sync.dma_start(
                                            bar_in[jj, :, :],
                                            last_on[0:1, 0:64])
                                    nc.gpsimd.collective_compute(
                                        "AllToAll", OP.bypass,
                                        replica_groups=[list(range(NCORES))],
                                        ins=[bar_in.opt()],
                                        outs=[bar_out.opt()],
                                    )
                        emit_collective_fetch(h)
                        if h == 0:
                            # wo is only needed at the output projection; fetch it
                            # during SDPA when HBM is otherwise quiet
                            nc.scalar.dma_start(wo_sb[:], wo_t[:])

                # ---------- phase D: output projection, split per head ----------
                # head-0's half runs while head-1's AllToAll is in flight
                with (
                    tc.tile_pool(name="ysb", bufs=1) as ypool,
                    tc.tile_pool(name="ysum", bufs=4) as ysumpool,
                    tc.tile_pool(name="yps", bufs=2, space="PSUM") as yppool,
                ):
                    # ordering anchor: keep the output-projection chains after
                    # the SDPA tail in the PE stream — the scheduler's cost
                    # model undershoots the AllToAll latency and would
                    # otherwise front-run these and stall the PE
                    anchor = mm_hold[0]
                    for hh in range(HL):
                        if hh == 1:
                            # bridge a possible wait for the second AllToAll
                            # with a few throwaway matmuls (the first real
                            # start=True clears the bank). Anchor them on the
                            # last h0 chain's FINAL matmul so they cannot
                            # interleave into that chain.
                            anchor = last_mm
                            warm_yp = yppool.tile([128, 512], F32, tag="yps",
                                                  name="warmyp")
                            for _ in range(12):
                                wmm = nc.tensor.matmul(
                                    warm_yp[:, 0:128], ones_sb[:], ones_sb[:],
                                    start=True, stop=True)
                                bass_rust.add_dep_helper(
                                    wmm.ins, anchor.ins, sync=False,
                                    reason="keep PE warm across a2a#1 wait")
                                anchor = wmm
                        for tt in range(TL // 128):
                            for eo in range(4):
                                yp = yppool.tile([128, 512], F32, tag="yps",
                                                 name="ypsum")
                                for di in range(NCORES):
                                    d = di * HL + hh
                                    mm = nc.tensor.matmul(
                                        yp[:],
                                        ot_sb[hh][:, di * TL + tt * 128:
                                                  di * TL + (tt + 1) * 128],
                                        wo_sb[:, d * D + eo * 512:
                                              d * D + (eo + 1) * 512],
                                        start=(di == 0), stop=(di == NCORES - 1))
                                    if di == 0:
                                        bass_rust.add_dep_helper(
                                            mm.ins, anchor.ins, sync=False,
                                            reason="order oproj after prior phase")
                                        anchor = mm
                                    last_mm = mm
                                if hh == 0:
                                    y0 = ypool.tile([128, 512], F32,
                                                    tag=f"y0_{tt}_{eo}",
                                                    name=f"y0_{tt}_{eo}")
                                    nc.vector.tensor_copy(y0[:], yp[:])
                                    ot_sb[(0, tt, eo)] = y0
                                else:
                                    ys = ysumpool.tile([128, 512], F32, tag="ysum",
                                                       name="ysum")
                                    nc.vector.tensor_tensor(
                                        ys[:], yp[:], ot_sb[(0, tt, eo)][:], OP.add)
                                    nc.sync.dma_start(
                                        out[tt * 128:(tt + 1) * 128,
                                            eo * 512:(eo + 1) * 512], ys[:])


    nc.compile()
    return nc


def _prep_in_maps(x, cos, sin, attn_mask, wq, wk, wv, wo):
    x_t = np.ascontiguousarray(
        np.asarray(x, np.float32).reshape(T, D).T.astype(bf16))      # [D, T]
    cosT = np.ascontiguousarray(np.asarray(cos[0], np.float32).T)    # [HD, S]
    sinT = np.asarray(sin[0], np.float32).T
    sin_m = np.ascontiguousarray(
        np.concatenate([-sinT[:64], sinT[64:]], axis=0))             # [HD, S]
    mask_t = np.ascontiguousarray(
        np.asarray(attn_mask, np.float32).reshape(B * NKT, 128).T)   # [128, 32]

    def pack(w_sl):
        # [E_out, D] slice -> [128, NDT * E_out] d-tile-major layout
        e_out = w_sl.shape[0]
        return np.ascontiguousarray(
            w_sl.T.reshape(NDT, 128, e_out).transpose(1, 0, 2)
            .reshape(128, NDT * e_out).astype(bf16))

    wo_t = pack(np.asarray(wo, np.float32))
    in_maps = []
    for i in range(NCORES):
        sl = slice(i * EL, (i + 1) * EL)
        in_maps.append({
            "x_t": x_t,
            "wq_t": pack(np.asarray(wq, np.float32)[sl]),
            "wk_t": pack(np.asarray(wk, np.float32)[sl]),
            "wv_t": pack(np.asarray(wv, np.float32)[sl]),
            "wo_t": wo_t,
            "cos_t": cosT.astype(bf16),
            "sin_m": sin_m.astype(bf16),
            "mask_t": mask_t,
        })
    return in_maps


def kernel(x, cos, sin, attn_mask, wq, wk, wv, wo, _trace=False):
    if "nc" not in _CACHE:
        _CACHE["nc"] = _build()
    nc = _CACHE["nc"]
    in_maps = _prep_in_maps(x, cos, sin, attn_mask, wq, wk, wv, wo)
    res = run_bass_kernel_spmd(nc, in_maps, core_ids=list(range(NCORES)),
                               trace=_trace)
    _CACHE["last_result"] = res
    y = np.concatenate([np.asarray(res.results[i]["out"], np.float32)
                        for i in range(NCORES)], axis=0)
    return y.reshape(B, S, D)



# revision 31
# speedup vs baseline: 1.0230x; 1.0228x over previous
"""Trainium2 Bass kernel for multi-head attention with RoPE (B=2, S=2048,
D=2048, H=16), distributed over 8 NeuronCores with head tensor-parallelism
and an AllToAll to switch to token-parallelism for the output projection.

kernel(**inputs) takes the full unsharded inputs (as produced by the
reference setup_inputs) and returns the full [2, 2048, 2048] f32 output.

Layout strategy: x is pre-transposed/cast to bf16 [D, T] on the host (same
spirit as the host-side weight transposes), so QKV matmuls stream straight
from SBUF xT tiles with no on-device staging. V is produced directly in
natural [t, hd] layout by swapping matmul operands. The output projection
is split into per-head halves so head-0's half overlaps the second
AllToAll.
"""
import numpy as np
import ml_dtypes
import bass_rust
from concourse import bass, bacc, tile, mybir
from concourse.bass_utils import run_bass_kernel_spmd

bf16 = ml_dtypes.bfloat16
BF16 = mybir.dt.bfloat16
F32 = mybir.dt.float32
AF = mybir.ActivationFunctionType
OP = mybir.AluOpType

B, S, D, H = 2, 2048, 2048, 16
HD = 128                 # head dim
NCORES = 8
HL = H // NCORES         # heads per core = 2
EL = HL * HD             # local projection width = 256
T = B * S                # 4096 flattened tokens
NG = 4                   # 1024-token groups in QKV phase
TG = T // NG             # 1024
NKT = S // 128           # 16 key tiles per batch
NQC = S // 512           # 4 query chunks per batch
NDT = D // 128           # 16 contraction tiles
TL = T // NCORES         # 512 tokens per core after AllToAll
SCALE = float(1.0 / np.sqrt(128.0))

_CACHE = {}


def _build():
    nc = bacc.Bacc("TRN2", target_bir_lowering=False, num_devices=NCORES)

    x_t = nc.dram_tensor("x_t", [D, T], BF16, kind="ExternalInput")
    wq_t = nc.dram_tensor("wq_t", [128, NDT * EL], BF16, kind="ExternalInput")
    wk_t = nc.dram_tensor("wk_t", [128, NDT * EL], BF16, kind="ExternalInput")
    wv_t = nc.dram_tensor("wv_t", [128, NDT * EL], BF16, kind="ExternalInput")
    wo_t = nc.dram_tensor("wo_t", [128, NDT * D], BF16, kind="ExternalInput")
    cos_t = nc.dram_tensor("cos_t", [HD, S], BF16, kind="ExternalInput")
    sin_m = nc.dram_tensor("sin_m", [HD, S], BF16, kind="ExternalInput")
    mask_t = nc.dram_tensor("mask_t", [128, B * NKT], F32, kind="ExternalInput")
    out = nc.dram_tensor("out", [TL, D], F32, kind="ExternalOutput")

    ones_dram = nc.inline_tensor(np.ones((128, 128), dtype=bf16), name="ones")

    with tile.TileContext(nc) as tc:
        with (
            tc.tile_pool(name="dram", bufs=1, space="DRAM") as dram,
            tc.tile_pool(name="consts", bufs=1) as consts,
            tc.tile_pool(name="keep", bufs=1) as keep,
        ):
            # AllToAll buffers. Head 0: ONE collective with full TL-wide
            # shards — 1KB contiguous lines halve the SDMA descriptor count
            # (descriptor rate ~8ns/512B caps split buffers at ~60GB/s) and
            # h0 has ~25us of slack anyway. Head 1 is on the critical path:
            # 2-way token split so the second half wire-pipelines under the
            # output projection of the first.
            a2a_in0 = dram.tile([NCORES, HD, TL], BF16,
                                tag="a2a_in0", name="a2a_in0")
            a2a_out0 = dram.tile([NCORES, HD, TL], BF16,
                                 tag="a2a_out0", name="a2a_out0")
            a2a_in1 = [dram.tile([NCORES, HD, TL // 2], BF16,
                                 tag=f"a2a_in1_{hf}", name=f"a2a_in1_{hf}")
                       for hf in range(2)]
            a2a_out1 = [dram.tile([NCORES, HD, TL // 2], BF16,
                                  tag=f"a2a_out1_{hf}", name=f"a2a_out1_{hf}")
                        for hf in range(2)]

            ones_sb = consts.tile([128, 128], BF16, tag="ones", name="ones_sb")
            nc.scalar.dma_start(ones_sb[:], ones_dram[:])
            mask_sb = consts.tile([128, B * NKT], F32, tag="mask", name="mask_sb")
            nc.scalar.dma_start(mask_sb[:], mask_t[:])
            cos_sb = consts.tile([128, S], BF16, tag="cos", name="cos_sb")
            sin_sb = consts.tile([128, S], BF16, tag="sin", name="sin_sb")

            # persistent per-head tensors: qT/kT in [hd, t]; v natural packed
            # per 128-token block as [t=128, (eh, hd)] along the free dim
            qT = [keep.tile([128, T], BF16, tag=f"qT{h}", name=f"qT{h}")
                  for h in range(HL)]
            kT = [keep.tile([128, T], BF16, tag=f"kT{h}", name=f"kT{h}")
                  for h in range(HL)]
            vnat = keep.tile([128, 2 * T], BF16, tag="vnat", name="vnat")

            # ---------- phase A+B: QKV projections + RoPE ----------
            with (
                tc.tile_pool(name="wsb", bufs=1) as wpool,
                tc.tile_pool(name="xt", bufs=32) as xtpool,
                tc.tile_pool(name="rope", bufs=3) as rope,
                tc.tile_pool(name="qkps", bufs=8, space="PSUM") as qkps,
            ):
                wsb = {}
                for nm, wt in (("q", wq_t), ("k", wk_t), ("v", wv_t)):
                    wsb[nm] = wpool.tile([128, NDT * EL], BF16, tag=f"w{nm}",
                                         name=f"w{nm}")

                # throwaway stationary/moving for PE warmups: memset (DVE)
                # instead of a DMA so the PE can start before any HBM
                # traffic lands
                warm_sb = wpool.tile([128, 128], BF16, tag="warm", name="warm")
                nc.vector.memset(warm_sb[:], 0)

                def emit_rope(ps, nm, eh, t0):
                    pos0 = t0 % S
                    dst = qT[eh] if nm == "q" else kT[eh]
                    tmp = rope.tile([128, 512], F32, tag="ropetmp",
                                    name="ropetmp")
                    nc.vector.tensor_tensor(
                        tmp[:], ps[:], cos_sb[:, pos0:pos0 + 512],
                        OP.mult)
                    u = rope.tile([128, 512], F32, tag="ropeu",
                                  name="ropeu")
                    nc.vector.tensor_tensor(
                        u[0:64, :], ps[64:128, :],
                        sin_sb[0:64, pos0:pos0 + 512], OP.mult)
                    nc.vector.tensor_tensor(
                        u[64:128, :], ps[0:64, :],
                        sin_sb[64:128, pos0:pos0 + 512], OP.mult)
                    nc.vector.tensor_tensor(
                        dst[:, t0:t0 + 512], tmp[:], u[:], OP.add)

                for g in range(NG):
                    g0 = g * TG
                    xts = []
                    for dti in range(NDT):
                        xtile = xtpool.tile([128, TG], BF16, tag="xt", name="xt")
                        if g == 0:
                            # interleave wq AND wk eighth-pairs with the
                            # first group's xt stream: the chunk covering
                            # chains (dti, dti+1) lands before xt[dti], and
                            # the per-slot DMA load stays smooth (~0.4MB) so
                            # tile arrivals pace evenly against PE work
                            if dti % 2 == 0:
                                c0 = (dti // 2) * (NDT * EL // 8)
                                c1 = (dti // 2 + 1) * (NDT * EL // 8)
                                nc.sync.dma_start(wsb["q"][:, c0:c1],
                                                  wq_t[:, c0:c1])
                                nc.sync.dma_start(wsb["k"][:, c0:c1],
                                                  wk_t[:, c0:c1])
                        xq = nc.scalar if (g == 0 and dti % 2 == 1) else nc.sync
                        xq.dma_start(
                            xtile[:], x_t[dti * 128:(dti + 1) * 128, g0:g0 + TG])
                        xts.append(xtile)
                        if g == 0 and dti == NDT - 1:
                            # cos/sin (rope tables, first used at the g0 chain
                            # tail ~30us in) and wv (~45us) stay out of the
                            # startup-critical DMA prefix
                            nc.sync.dma_start(cos_sb[:], cos_t[:])
                            nc.sync.dma_start(sin_sb[:], sin_m[:])
                            nc.sync.dma_start(wsb["v"][:], wv_t[:])
                    if g == 0:
                        # group 0 is paced by the xt/w DMA stream: run all 8
                        # chains (q/k x head x half) d-outer through dti 11 so
                        # each arriving xt tile unlocks 8 matmuls (~3.4us of
                        # cold-clock PE work vs ~1.5us DMA spacing); then
                        # finish the chains ONE AT A TIME (dti 12-15 + rope)
                        # so the chain stops stagger and the rope DVE burst
                        # pipelines into the V phase instead of gating it
                        chains = [(nm, eh, half) for nm in ("q", "k")
                                  for eh in range(HL) for half in range(2)]
                        pss = {c: qkps.tile([128, 512], F32, tag="qkps",
                                            name="qkps") for c in chains}
                        # bridge the ~2.8us before the first xt tile lands
                        # (keeps HAM warm); the first real start=True matmul
                        # clears this bank
                        for _ in range(26):
                            nc.tensor.matmul(
                                pss[("q", 0, 0)][:, 0:128],
                                warm_sb[:], warm_sb[:],
                                start=True, stop=True)
                        for dti in range(12):
                            for nm, eh, half in chains:
                                nc.tensor.matmul(
                                    pss[(nm, eh, half)][:],
                                    wsb[nm][:, dti * EL + eh * 128:
                                            dti * EL + (eh + 1) * 128],
                                    xts[dti][:, half * 512:(half + 1) * 512],
                                    start=(dti == 0), stop=False)
                        for nm, eh, half in chains:
                            for dti in range(12, NDT):
                                nc.tensor.matmul(
                                    pss[(nm, eh, half)][:],
                                    wsb[nm][:, dti * EL + eh * 128:
                                            dti * EL + (eh + 1) * 128],
                                    xts[dti][:, half * 512:(half + 1) * 512],
                                    start=False, stop=(dti == NDT - 1))
                            emit_rope(pss[(nm, eh, half)], nm, eh,
                                      g0 + half * 512)
                    else:
                        # steady state: half-major emission keeps each half's
                        # rope (DVE) hidden under the next half's matmuls
                        for half in range(2):
                            t0 = g0 + half * 512
                            for nm, eh in [(nm, eh) for nm in ("q", "k")
                                           for eh in range(HL)]:
                                ps = qkps.tile([128, 512], F32, tag="qkps",
                                               name="qkps")
                                for dti in range(NDT):
                                    nc.tensor.matmul(
                                        ps[:],
                                        wsb[nm][:, dti * EL + eh * 128:
                                                dti * EL + (eh + 1) * 128],
                                        xts[dti][:, half * 512:(half + 1) * 512],
                                        start=(dti == 0), stop=(dti == NDT - 1))
                                emit_rope(ps, nm, eh, t0)
                    for tb in range(TG // 128):
                        t0 = g0 + tb * 128
                        # V psum shares the qkps ring (padded to the same
                        # [128,512] slot) so the QKV phase fits in 8 banks
                        ps = qkps.tile([128, EL], F32, tag="qkps",
                                       padded_shape=[128, 512], name="vps")
                        for dti in range(NDT):
                            nc.tensor.matmul(
                                ps[:],
                                xts[dti][:, tb * 128:(tb + 1) * 128],
                                wsb["v"][:, dti * EL:(dti + 1) * EL],
                                start=(dti == 0), stop=(dti == NDT - 1))
                        nc.vector.tensor_copy(
                            vnat[:, t0 * 2:t0 * 2 + EL], ps[:])

            # ---------- phase C: SDPA per (head, batch, 1024-query block) ----------
            ot_sb = {}
            mm_hold = [None]
            with (
                tc.tile_pool(name="late", bufs=1) as late,
                # small SBUF pools that outlive the SDPA PSUM scope: the
                # FINAL block's denominator/normalize/a2a are emitted inside
                # the oproj phase so the PE never idles at the transition
                tc.tile_pool(name="onorm", bufs=6) as onpool,
                tc.tile_pool(name="rec", bufs=3) as recpool,
            ):
                wo_sb = late.tile([128, NDT * D], BF16, tag="wo", name="wo_sb")
                otpool = late

                bar_src = [None]

                def emit_norm_a2a(h, b, qp, get_den, get_ops):
                    for qc2 in range(2):
                        rec = recpool.tile([128, 512], F32, tag="rec",
                                           name="rec")
                        nc.vector.reciprocal_approx_fast(
                            rec[:], get_den(qc2))
                        on = onpool.tile([128, 512], BF16, tag="on",
                                         name="onorm")
                        nc.vector.tensor_tensor(
                            on[:], get_ops(qc2), rec[:], OP.mult)
                        bar_src[0] = on
                        j = b * NQC + qp * 2 + qc2
                        if h == 0:
                            nc.sync.dma_start(a2a_in0[j, :, :], on[:])
                        else:
                            for hf in range(2):
                                nc.sync.dma_start(
                                    a2a_in1[hf][j, :, :],
                                    on[:, hf * 256:(hf + 1) * 256])

                def emit_collective_fetch(h):
                    # fetches ride the sync ring: NOT the scalar ring
                    # (blocks ScalarE's exp stream) and NOT the gpsimd ring
                    # (measured: Pool-queue DMAs wedge the collective train
                    # — op0 went 26us -> 113us)
                    otile = otpool.tile([128, NCORES * TL], BF16,
                                        tag=f"ot{h}", name=f"ot{h}")
                    if h == 0:
                        nc.gpsimd.collective_compute(
                            "AllToAll", OP.bypass,
                            replica_groups=[list(range(NCORES))],
                            ins=[a2a_in0.opt()],
                            outs=[a2a_out0.opt()],
                        )
                        for src in range(NCORES):
                            nc.sync.dma_start(
                                otile[:, src * TL:(src + 1) * TL],
                                a2a_out0[src, :, :])
                    else:
                        for hf in range(2):
                            nc.gpsimd.collective_compute(
                                "AllToAll", OP.bypass,
                                replica_groups=[list(range(NCORES))],
                                ins=[a2a_in1[hf].opt()],
                                outs=[a2a_out1[hf].opt()],
                            )
                        for hf in range(2):
                            for src in range(NCORES):
                                # alternate queues: ScalarE's exp stream is
                                # done by the time these can run, and two
                                # rings double the descriptor rate on these
                                # 512B-line transfers
                                fq = nc.sync if src % 2 == 0 else nc.scalar
                                fq.dma_start(
                                    otile[:, src * TL + hf * 256:
                                          src * TL + (hf + 1) * 256],
                                    a2a_out1[hf][src, :, :])
                    ot_sb[h] = otile

                with (
                    tc.tile_pool(name="E", bufs=16) as epool,
                    tc.tile_pool(name="Epair", bufs=4) as eppool,
                    tc.tile_pool(name="Equad", bufs=3) as eqpool,
                    tc.tile_pool(name="sps", bufs=2, space="PSUM") as spool,
                    tc.tile_pool(name="ops", bufs=2, space="PSUM") as opool,
                ):
                    for h in range(HL):
                        for b in range(B):
                            q0 = b * S
                            for qp in range(2):
                                qb = q0 + qp * 1024
                                last_block = (h == HL - 1 and b == B - 1
                                              and qp == 1)
                                ops_ps = [opool.tile([128, 512], F32, tag="ops",
                                                     name="opsum")
                                          for _ in range(2)]
                                E = []
                                pairs = []
                                quads = []
                                octs = []

                                def attn_step(kt):
                                    e_t = E[kt]
                                    vcol = (b * NKT + kt) * EL + h * 128
                                    for qc2 in range(2):
                                        mm_hold[0] = nc.tensor.matmul(
                                            ops_ps[qc2][:],
                                            vnat[:, vcol:vcol + 128],
                                            e_t[:, qc2 * 512:(qc2 + 1) * 512],
                                            start=(kt == 0), stop=(kt == NKT - 1))
                                    if kt % 2 == 1:
                                        ep = eppool.tile([128, 1024], BF16,
                                                         tag="epair", name="epair")
                                        nc.vector.tensor_tensor(
                                            ep[:], E[kt - 1][:], e_t[:], OP.add)
                                        pairs.append(ep)
                                    if kt % 4 == 3:
                                        eq = eqpool.tile([128, 1024], BF16,
                                                         tag="equad", name="equad")
                                        nc.vector.tensor_tensor(
                                            eq[:], pairs[-2][:], pairs[-1][:],
                                            OP.add)
                                        quads.append(eq)
                                    if kt % 8 == 7:
                                        eo8 = eqpool.tile([128, 1024], BF16,
                                                          tag="eoct", name="eoct")
                                        nc.vector.tensor_tensor(
                                            eo8[:], quads[-2][:], quads[-1][:],
                                            OP.add)
                                        octs.append(eo8)

                                for kt in range(NKT):
                                    sp = spool.tile([128, 1024], F32, tag="sps",
                                                    name="spsum")
                                    for qc2 in range(2):
                                        nc.tensor.matmul(
                                            sp[:, qc2 * 512:(qc2 + 1) * 512],
                                            kT[h][:, q0 + kt * 128:
                                                  q0 + (kt + 1) * 128],
                                            qT[h][:, qb + qc2 * 512:
                                                  qb + (qc2 + 1) * 512],
                                            start=True, stop=True)
                                    e_t = epool.tile([128, 1024], BF16, tag="E",
                                                     name="etile")
                                    mcol = b * NKT + kt
                                    nc.scalar.activation(
                                        e_t[:], sp[:], AF.Exp,
                                        bias=mask_sb[:, mcol:mcol + 1],
                                        scale=SCALE)
                                    E.append(e_t)
                                    # lag the PV/denominator consumption FOUR
                                    # score tiles behind the exp producer: the
                                    # 8-matmul PV backlog at the block tail
                                    # then covers the exp->tree->dps->norm
                                    # serial chain, so the next block's PV(0)
                                    # never stalls on the ops-psum release.
                                    # The FINAL block instead uses lag 2: no
                                    # next block exists, and a short tail gets
                                    # the last AllToAll triggered ~3us sooner
                                    # (the trigger is on the critical path).
                                    lag = 2 if last_block else 4
                                    if kt >= lag:
                                        attn_step(kt - lag)
                                for kt in range(NKT - lag, NKT):
                                    attn_step(kt)

                                # full 16-level tree: one ones-matmul pair per
                                # block (not per oct); its psum has its own
                                # 2-bank slot so the scores ring never waits
                                # on the rec/norm chain
                                e16 = eqpool.tile([128, 1024], BF16,
                                                  tag="eoct", name="e16")
                                nc.vector.tensor_tensor(
                                    e16[:], octs[-2][:], octs[-1][:], OP.add)
                                dps_ps = spool.tile([128, 1024], F32,
                                                    tag="dps", bufs=1,
                                                    name="dpsum")
                                for qc2 in range(2):
                                    mm_hold[0] = nc.tensor.matmul(
                                        dps_ps[:, qc2 * 512:(qc2 + 1) * 512],
                                        ones_sb[:],
                                        e16[:, qc2 * 512:(qc2 + 1) * 512],
                                        start=True, stop=True)
                                if last_block:
                                    # free the ops psum early: the norm reads
                                    # an SBUF copy, so the oproj psum pool's
                                    # boundary only waits on the dps recs
                                    opc = [onpool.tile([128, 512], F32,
                                                       tag="opscopy", bufs=2,
                                                       name="opsc")
                                           for _ in range(2)]
                                    for qc2 in range(2):
                                        nc.vector.tensor_copy(
                                            opc[qc2][:], ops_ps[qc2][:])
                                    emit_norm_a2a(
                                        h, b, qp,
                                        lambda qc2: dps_ps[:, qc2 * 512:
                                                           (qc2 + 1) * 512],
                                        lambda qc2: opc[qc2][:])
                                else:
                                    emit_norm_a2a(
                                        h, b, qp,
                                        lambda qc2: dps_ps[:, qc2 * 512:
                                                           (qc2 + 1) * 512],
                                        lambda qc2: ops_ps[qc2][:])
                        emit_collective_fetch(h)
                        if h == 0:
                            # wo is only needed at the output projection; fetch it
                            # during SDPA when HBM is otherwise quiet
                            nc.scalar.dma_start(wo_sb[:], wo_t[:])

                # ---------- phase D: output projection, split per head ----------
                # head-0's half runs while head-1's AllToAll is in flight
                with (
                    tc.tile_pool(name="ysb", bufs=1) as ypool,
                    tc.tile_pool(name="ysum", bufs=4) as ysumpool,
                    tc.tile_pool(name="yps", bufs=2, space="PSUM") as yppool,
                ):
                    # ordering anchor: keep the output-projection chains after
                    # the SDPA tail in the PE stream — the scheduler's cost
                    # model undershoots the AllToAll latency and would
                    # otherwise front-run these and stall the PE
                    anchor = mm_hold[0]
                    for hh in range(HL):
                        if hh == 1:
                            # bridge a possible wait for the second AllToAll
                            # with a few throwaway matmuls (the first real
                            # start=True clears the bank). Anchor them on the
                            # last h0 chain's FINAL matmul so they cannot
                            # interleave into that chain.
                            anchor = last_mm
                            warm_yp = yppool.tile([128, 512], F32, tag="yps",
                                                  name="warmyp")
                            for _ in range(12):
                                wmm = nc.tensor.matmul(
                                    warm_yp[:, 0:128], ones_sb[:], ones_sb[:],
                                    start=True, stop=True)
                                bass_rust.add_dep_helper(
                                    wmm.ins, anchor.ins, sync=False,
                                    reason="keep PE warm across a2a#1 wait")
                                anchor = wmm
                        for tt in range(TL // 128):
                            for eo in range(4):
                                yp = yppool.tile([128, 512], F32, tag="yps",
                                                 name="ypsum")
                                for di in range(NCORES):
                                    d = di * HL + hh
                                    mm = nc.tensor.matmul(
                                        yp[:],
                                        ot_sb[hh][:, di * TL + tt * 128:
                                                  di * TL + (tt + 1) * 128],
                                        wo_sb[:, d * D + eo * 512:
                                              d * D + (eo + 1) * 512],
                                        start=(di == 0), stop=(di == NCORES - 1))
                                    if di == 0:
                                        bass_rust.add_dep_helper(
                                            mm.ins, anchor.ins, sync=False,
                                            reason="order oproj after prior phase")
                                        anchor = mm
                                    last_mm = mm
                                if hh == 0:
                                    y0 = ypool.tile([128, 512], F32,
                                                    tag=f"y0_{tt}_{eo}",
                                                    name=f"y0_{tt}_{eo}")
                                    nc.vector.tensor_copy(y0[:], yp[:])
                                    ot_sb[(0, tt, eo)] = y0
                                else:
                                    ys = ysumpool.tile([128, 512], F32, tag="ysum",
                                                       name="ysum")
                                    nc.vector.tensor_tensor(
                                        ys[:], yp[:], ot_sb[(0, tt, eo)][:], OP.add)
                                    nc.sync.dma_start(
                                        out[tt * 128:(tt + 1) * 128,
                                            eo * 512:(eo + 1) * 512], ys[:])


    nc.compile()
    return nc


def _prep_in_maps(x, cos, sin, attn_mask, wq, wk, wv, wo):
    x_t = np.ascontiguousarray(
        np.asarray(x, np.float32).reshape(T, D).T.astype(bf16))      # [D, T]
    cosT = np.ascontiguousarray(np.asarray(cos[0], np.float32).T)    # [HD, S]
    sinT = np.asarray(sin[0], np.float32).T
    sin_m = np.ascontiguousarray(
        np.concatenate([-sinT[:64], sinT[64:]], axis=0))             # [HD, S]
    mask_t = np.ascontiguousarray(
        np.asarray(attn_mask, np.float32).reshape(B * NKT, 128).T)   # [128, 32]

    def pack(w_sl):
        # [E_out, D] slice -> [128, NDT * E_out] d-tile-major layout
        e_out = w_sl.shape[0]
        return np.ascontiguousarray(
            w_sl.T.reshape(NDT, 128, e_out).transpose(1, 0, 2)
            .reshape(128, NDT * e_out).astype(bf16))

    wo_t = pack(np.asarray(wo, np.float32))
    in_maps = []
    for i in range(NCORES):
        sl = slice(i * EL, (i + 1) * EL)
        in_maps.append({
            "x_t": x_t,
            "wq_t": pack(np.asarray(wq, np.float32)[sl]),
            "wk_t": pack(np.asarray(wk, np.float32)[sl]),
            "wv_t": pack(np.asarray(wv, np.float32)[sl]),
            "wo_t": wo_t,
            "cos_t": cosT.astype(bf16),
            "sin_m": sin_m.astype(bf16),
            "mask_t": mask_t,
        })
    return in_maps


def kernel(x, cos, sin, attn_mask, wq, wk, wv, wo, _trace=False):
    if "nc" not in _CACHE:
        _CACHE["nc"] = _build()
    nc = _CACHE["nc"]
    in_maps = _prep_in_maps(x, cos, sin, attn_mask, wq, wk, wv, wo)
    res = run_bass_kernel_spmd(nc, in_maps, core_ids=list(range(NCORES)),
                               trace=_trace)
    _CACHE["last_result"] = res
    y = np.concatenate([np.asarray(res.results[i]["out"], np.float32)
                        for i in range(NCORES)], axis=0)
    return y.reshape(B, S, D)



# revision 32
# speedup vs baseline: 1.0232x; 1.0002x over previous
"""Trainium2 Bass kernel for multi-head attention with RoPE (B=2, S=2048,
D=2048, H=16), distributed over 8 NeuronCores with head tensor-parallelism
and an AllToAll to switch to token-parallelism for the output projection.

kernel(**inputs) takes the full unsharded inputs (as produced by the
reference setup_inputs) and returns the full [2, 2048, 2048] f32 output.

Layout strategy: x is pre-transposed/cast to bf16 [D, T] on the host (same
spirit as the host-side weight transposes), so QKV matmuls stream straight
from SBUF xT tiles with no on-device staging. V is produced directly in
natural [t, hd] layout by swapping matmul operands. The output projection
is split into per-head halves so head-0's half overlaps the second
AllToAll.
"""
import numpy as np
import ml_dtypes
import bass_rust
from concourse import bass, bacc, tile, mybir
from concourse.bass_utils import run_bass_kernel_spmd

bf16 = ml_dtypes.bfloat16
BF16 = mybir.dt.bfloat16
F32 = mybir.dt.float32
AF = mybir.ActivationFunctionType
OP = mybir.AluOpType

B, S, D, H = 2, 2048, 2048, 16
HD = 128                 # head dim
NCORES = 8
HL = H // NCORES         # heads per core = 2
EL = HL * HD             # local projection width = 256
T = B * S                # 4096 flattened tokens
NG = 4                   # 1024-token groups in QKV phase
TG = T // NG             # 1024
NKT = S // 128           # 16 key tiles per batch
NQC = S // 512           # 4 query chunks per batch
NDT = D // 128           # 16 contraction tiles
TL = T // NCORES         # 512 tokens per core after AllToAll
SCALE = float(1.0 / np.sqrt(128.0))

_CACHE = {}


def _build():
    nc = bacc.Bacc("TRN2", target_bir_lowering=False, num_devices=NCORES)

    x_t = nc.dram_tensor("x_t", [D, T], BF16, kind="ExternalInput")
    wq_t = nc.dram_tensor("wq_t", [128, NDT * EL], BF16, kind="ExternalInput")
    wk_t = nc.dram_tensor("wk_t", [128, NDT * EL], BF16, kind="ExternalInput")
    wv_t = nc.dram_tensor("wv_t", [128, NDT * EL], BF16, kind="ExternalInput")
    wo_t = nc.dram_tensor("wo_t", [128, NDT * D], BF16, kind="ExternalInput")
    cos_t = nc.dram_tensor("cos_t", [HD, S], BF16, kind="ExternalInput")
    sin_m = nc.dram_tensor("sin_m", [HD, S], BF16, kind="ExternalInput")
    mask_t = nc.dram_tensor("mask_t", [128, B * NKT], F32, kind="ExternalInput")
    out = nc.dram_tensor("out", [TL, D], F32, kind="ExternalOutput")

    ones_dram = nc.inline_tensor(np.ones((128, 128), dtype=bf16), name="ones")

    with tile.TileContext(nc) as tc:
        with (
            tc.tile_pool(name="dram", bufs=1, space="DRAM") as dram,
            tc.tile_pool(name="consts", bufs=1) as consts,
            tc.tile_pool(name="keep", bufs=1) as keep,
        ):
            # AllToAll buffers. Head 0: ONE collective with full TL-wide
            # shards — 1KB contiguous lines halve the SDMA descriptor count
            # (descriptor rate ~8ns/512B caps split buffers at ~60GB/s) and
            # h0 has ~25us of slack anyway. Head 1 is on the critical path:
            # 2-way token split so the second half wire-pipelines under the
            # output projection of the first.
            a2a_in0 = dram.tile([NCORES, HD, TL], BF16,
                                tag="a2a_in0", name="a2a_in0")
            a2a_out0 = dram.tile([NCORES, HD, TL], BF16,
                                 tag="a2a_out0", name="a2a_out0")
            a2a_in1 = [dram.tile([NCORES, HD, TL // 2], BF16,
                                 tag=f"a2a_in1_{hf}", name=f"a2a_in1_{hf}")
                       for hf in range(2)]
            a2a_out1 = [dram.tile([NCORES, HD, TL // 2], BF16,
                                  tag=f"a2a_out1_{hf}", name=f"a2a_out1_{hf}")
                        for hf in range(2)]

            ones_sb = consts.tile([128, 128], BF16, tag="ones", name="ones_sb")
            nc.scalar.dma_start(ones_sb[:], ones_dram[:])
            mask_sb = consts.tile([128, B * NKT], F32, tag="mask", name="mask_sb")
            nc.scalar.dma_start(mask_sb[:], mask_t[:])
            cos_sb = consts.tile([128, S], BF16, tag="cos", name="cos_sb")
            sin_sb = consts.tile([128, S], BF16, tag="sin", name="sin_sb")

            # persistent per-head tensors: qT/kT in [hd, t]; v natural packed
            # per 128-token block as [t=128, (eh, hd)] along the free dim
            qT = [keep.tile([128, T], BF16, tag=f"qT{h}", name=f"qT{h}")
                  for h in range(HL)]
            kT = [keep.tile([128, T], BF16, tag=f"kT{h}", name=f"kT{h}")
                  for h in range(HL)]
            vnat = keep.tile([128, 2 * T], BF16, tag="vnat", name="vnat")

            # ---------- phase A+B: QKV projections + RoPE ----------
            with (
                tc.tile_pool(name="wsb", bufs=1) as wpool,
                tc.tile_pool(name="xt", bufs=32) as xtpool,
                tc.tile_pool(name="rope", bufs=3) as rope,
                tc.tile_pool(name="qkps", bufs=8, space="PSUM") as qkps,
            ):
                wsb = {}
                for nm, wt in (("q", wq_t), ("k", wk_t), ("v", wv_t)):
                    wsb[nm] = wpool.tile([128, NDT * EL], BF16, tag=f"w{nm}",
                                         name=f"w{nm}")

                # throwaway stationary/moving for PE warmups: memset (DVE)
                # instead of a DMA so the PE can start before any HBM
                # traffic lands
                warm_sb = wpool.tile([128, 128], BF16, tag="warm", name="warm")
                nc.vector.memset(warm_sb[:], 0)

                def emit_rope(ps, nm, eh, t0):
                    pos0 = t0 % S
                    dst = qT[eh] if nm == "q" else kT[eh]
                    tmp = rope.tile([128, 512], F32, tag="ropetmp",
                                    name="ropetmp")
                    nc.vector.tensor_tensor(
                        tmp[:], ps[:], cos_sb[:, pos0:pos0 + 512],
                        OP.mult)
                    u = rope.tile([128, 512], F32, tag="ropeu",
                                  name="ropeu")
                    nc.vector.tensor_tensor(
                        u[0:64, :], ps[64:128, :],
                        sin_sb[0:64, pos0:pos0 + 512], OP.mult)
                    nc.vector.tensor_tensor(
                        u[64:128, :], ps[0:64, :],
                        sin_sb[64:128, pos0:pos0 + 512], OP.mult)
                    nc.vector.tensor_tensor(
                        dst[:, t0:t0 + 512], tmp[:], u[:], OP.add)

                for g in range(NG):
                    g0 = g * TG
                    xts = []
                    for dti in range(NDT):
                        xtile = xtpool.tile([128, TG], BF16, tag="xt", name="xt")
                        if g == 0:
                            # interleave wq AND wk eighth-pairs with the
                            # first group's xt stream: the chunk covering
                            # chains (dti, dti+1) lands before xt[dti], and
                            # the per-slot DMA load stays smooth (~0.4MB) so
                            # tile arrivals pace evenly against PE work
                            if dti % 2 == 0:
                                c0 = (dti // 2) * (NDT * EL // 8)
                                c1 = (dti // 2 + 1) * (NDT * EL // 8)
                                nc.sync.dma_start(wsb["q"][:, c0:c1],
                                                  wq_t[:, c0:c1])
                                nc.sync.dma_start(wsb["k"][:, c0:c1],
                                                  wk_t[:, c0:c1])
                        xq = nc.scalar if (g == 0 and dti % 2 == 1) else nc.sync
                        xq.dma_start(
                            xtile[:], x_t[dti * 128:(dti + 1) * 128, g0:g0 + TG])
                        xts.append(xtile)
                        if g == 0 and dti == NDT - 1:
                            # cos/sin (rope tables, first used at the g0 chain
                            # tail ~30us in) and wv (~45us) stay out of the
                            # startup-critical DMA prefix
                            nc.sync.dma_start(cos_sb[:], cos_t[:])
                            nc.sync.dma_start(sin_sb[:], sin_m[:])
                            nc.sync.dma_start(wsb["v"][:], wv_t[:])
                    if g == 0:
                        # group 0 is paced by the xt/w DMA stream: run all 8
                        # chains (q/k x head x half) d-outer through dti 11 so
                        # each arriving xt tile unlocks 8 matmuls (~3.4us of
                        # cold-clock PE work vs ~1.5us DMA spacing); then
                        # finish the chains ONE AT A TIME (dti 12-15 + rope)
                        # so the chain stops stagger and the rope DVE burst
                        # pipelines into the V phase instead of gating it
                        chains = [(nm, eh, half) for nm in ("q", "k")
                                  for eh in range(HL) for half in range(2)]
                        pss = {c: qkps.tile([128, 512], F32, tag="qkps",
                                            name="qkps") for c in chains}
                        # bridge the ~2.8us before the first xt tile lands
                        # (keeps HAM warm); the first real start=True matmul
                        # clears this bank
                        for _ in range(26):
                            nc.tensor.matmul(
                                pss[("q", 0, 0)][:, 0:128],
                                warm_sb[:], warm_sb[:],
                                start=True, stop=True)
                        for dti in range(12):
                            for nm, eh, half in chains:
                                nc.tensor.matmul(
                                    pss[(nm, eh, half)][:],
                                    wsb[nm][:, dti * EL + eh * 128:
                                            dti * EL + (eh + 1) * 128],
                                    xts[dti][:, half * 512:(half + 1) * 512],
                                    start=(dti == 0), stop=False)
                        for nm, eh, half in chains:
                            for dti in range(12, NDT):
                                nc.tensor.matmul(
                                    pss[(nm, eh, half)][:],
                                    wsb[nm][:, dti * EL + eh * 128:
                                            dti * EL + (eh + 1) * 128],
                                    xts[dti][:, half * 512:(half + 1) * 512],
                                    start=False, stop=(dti == NDT - 1))
                            emit_rope(pss[(nm, eh, half)], nm, eh,
                                      g0 + half * 512)
                    else:
                        # steady state: half-major emission keeps each half's
                        # rope (DVE) hidden under the next half's matmuls
                        for half in range(2):
                            t0 = g0 + half * 512
                            for nm, eh in [(nm, eh) for nm in ("q", "k")
                                           for eh in range(HL)]:
                                ps = qkps.tile([128, 512], F32, tag="qkps",
                                               name="qkps")
                                for dti in range(NDT):
                                    nc.tensor.matmul(
                                        ps[:],
                                        wsb[nm][:, dti * EL + eh * 128:
                                                dti * EL + (eh + 1) * 128],
                                        xts[dti][:, half * 512:(half + 1) * 512],
                                        start=(dti == 0), stop=(dti == NDT - 1))
                                emit_rope(ps, nm, eh, t0)
                    for tb in range(TG // 128):
                        t0 = g0 + tb * 128
                        # V psum shares the qkps ring (padded to the same
                        # [128,512] slot) so the QKV phase fits in 8 banks
                        ps = qkps.tile([128, EL], F32, tag="qkps",
                                       padded_shape=[128, 512], name="vps")
                        for dti in range(NDT):
                            nc.tensor.matmul(
                                ps[:],
                                xts[dti][:, tb * 128:(tb + 1) * 128],
                                wsb["v"][:, dti * EL:(dti + 1) * EL],
                                start=(dti == 0), stop=(dti == NDT - 1))
                        nc.vector.tensor_copy(
                            vnat[:, t0 * 2:t0 * 2 + EL], ps[:])

            # ---------- phase C: SDPA per (head, batch, 1024-query block) ----------
            ot_sb = {}
            mm_hold = [None]
            with (
                tc.tile_pool(name="late", bufs=1) as late,
                # small SBUF pools that outlive the SDPA PSUM scope: the
                # FINAL block's denominator/normalize/a2a are emitted inside
                # the oproj phase so the PE never idles at the transition
                tc.tile_pool(name="onorm", bufs=6) as onpool,
                tc.tile_pool(name="rec", bufs=3) as recpool,
            ):
                wo_sb = late.tile([128, NDT * D], BF16, tag="wo", name="wo_sb")
                otpool = late

                def emit_norm_a2a(h, b, qp, get_den, get_ops):
                    for qc2 in range(2):
                        rec = recpool.tile([128, 512], F32, tag="rec",
                                           name="rec")
                        nc.vector.reciprocal_approx_fast(
                            rec[:], get_den(qc2))
                        on = onpool.tile([128, 512], BF16, tag="on",
                                         name="onorm")
                        nc.vector.tensor_tensor(
                            on[:], get_ops(qc2), rec[:], OP.mult)
                        j = b * NQC + qp * 2 + qc2
                        if h == 0:
                            nc.sync.dma_start(a2a_in0[j, :, :], on[:])
                        else:
                            for hf in range(2):
                                nc.sync.dma_start(
                                    a2a_in1[hf][j, :, :],
                                    on[:, hf * 256:(hf + 1) * 256])

                def emit_collective_fetch(h):
                    # fetches ride the sync ring: NOT the scalar ring
                    # (blocks ScalarE's exp stream) and NOT the gpsimd ring
                    # (measured: Pool-queue DMAs wedge the collective train
                    # — op0 went 26us -> 113us)
                    otile = otpool.tile([128, NCORES * TL], BF16,
                                        tag=f"ot{h}", name=f"ot{h}")
                    if h == 0:
                        nc.gpsimd.collective_compute(
                            "AllToAll", OP.bypass,
                            replica_groups=[list(range(NCORES))],
                            ins=[a2a_in0.opt()],
                            outs=[a2a_out0.opt()],
                        )
                        for src in range(NCORES):
                            nc.sync.dma_start(
                                otile[:, src * TL:(src + 1) * TL],
                                a2a_out0[src, :, :])
                    else:
                        for hf in range(2):
                            nc.gpsimd.collective_compute(
                                "AllToAll", OP.bypass,
                                replica_groups=[list(range(NCORES))],
                                ins=[a2a_in1[hf].opt()],
                                outs=[a2a_out1[hf].opt()],
                            )
                        for hf in range(2):
                            for src in range(NCORES):
                                # alternate queues: ScalarE's exp stream is
                                # done by the time these can run, and two
                                # rings double the descriptor rate on these
                                # 512B-line transfers
                                fq = nc.sync if src % 2 == 0 else nc.scalar
                                fq.dma_start(
                                    otile[:, src * TL + hf * 256:
                                          src * TL + (hf + 1) * 256],
                                    a2a_out1[hf][src, :, :])
                    ot_sb[h] = otile

                with (
                    tc.tile_pool(name="E", bufs=16) as epool,
                    tc.tile_pool(name="Epair", bufs=4) as eppool,
                    tc.tile_pool(name="Equad", bufs=3) as eqpool,
                    tc.tile_pool(name="sps", bufs=2, space="PSUM") as spool,
                    tc.tile_pool(name="ops", bufs=2, space="PSUM") as opool,
                ):
                    for h in range(HL):
                        for b in range(B):
                            q0 = b * S
                            for qp in range(2):
                                qb = q0 + qp * 1024
                                last_block = (h == HL - 1 and b == B - 1
                                              and qp == 1)
                                ops_ps = [opool.tile([128, 512], F32, tag="ops",
                                                     name="opsum")
                                          for _ in range(2)]
                                E = []
                                pairs = []
                                quads = []
                                octs = []

                                def attn_step(kt):
                                    e_t = E[kt]
                                    vcol = (b * NKT + kt) * EL + h * 128
                                    for qc2 in range(2):
                                        mm_hold[0] = nc.tensor.matmul(
                                            ops_ps[qc2][:],
                                            vnat[:, vcol:vcol + 128],
                                            e_t[:, qc2 * 512:(qc2 + 1) * 512],
                                            start=(kt == 0), stop=(kt == NKT - 1))
                                    if kt % 2 == 1:
                                        ep = eppool.tile([128, 1024], BF16,
                                                         tag="epair", name="epair")
                                        nc.vector.tensor_tensor(
                                            ep[:], E[kt - 1][:], e_t[:], OP.add)
                                        pairs.append(ep)
                                    if kt % 4 == 3:
                                        eq = eqpool.tile([128, 1024], BF16,
                                                         tag="equad", name="equad")
                                        nc.vector.tensor_tensor(
                                            eq[:], pairs[-2][:], pairs[-1][:],
                                            OP.add)
                                        quads.append(eq)
                                    if kt % 8 == 7:
                                        eo8 = eqpool.tile([128, 1024], BF16,
                                                          tag="eoct", name="eoct")
                                        nc.vector.tensor_tensor(
                                            eo8[:], quads[-2][:], quads[-1][:],
                                            OP.add)
                                        octs.append(eo8)

                                for kt in range(NKT):
                                    sp = spool.tile([128, 1024], F32, tag="sps",
                                                    name="spsum")
                                    for qc2 in range(2):
                                        nc.tensor.matmul(
                                            sp[:, qc2 * 512:(qc2 + 1) * 512],
                                            kT[h][:, q0 + kt * 128:
                                                  q0 + (kt + 1) * 128],
                                            qT[h][:, qb + qc2 * 512:
                                                  qb + (qc2 + 1) * 512],
                                            start=True, stop=True)
                                    e_t = epool.tile([128, 1024], BF16, tag="E",
                                                     name="etile")
                                    mcol = b * NKT + kt
                                    nc.scalar.activation(
                                        e_t[:], sp[:], AF.Exp,
                                        bias=mask_sb[:, mcol:mcol + 1],
                                        scale=SCALE)
                                    E.append(e_t)
                                    # lag the PV/denominator consumption FOUR
                                    # score tiles behind the exp producer: the
                                    # 8-matmul PV backlog at the block tail
                                    # then covers the exp->tree->dps->norm
                                    # serial chain, so the next block's PV(0)
                                    # never stalls on the ops-psum release.
                                    # The FINAL block instead uses lag 2: no
                                    # next block exists, and a short tail gets
                                    # the last AllToAll triggered ~3us sooner
                                    # (the trigger is on the critical path).
                                    lag = 2 if last_block else 4
                                    if kt >= lag:
                                        attn_step(kt - lag)
                                for kt in range(NKT - lag, NKT):
                                    attn_step(kt)

                                # full 16-level tree: one ones-matmul pair per
                                # block (not per oct); its psum has its own
                                # 2-bank slot so the scores ring never waits
                                # on the rec/norm chain
                                e16 = eqpool.tile([128, 1024], BF16,
                                                  tag="eoct", name="e16")
                                nc.vector.tensor_tensor(
                                    e16[:], octs[-2][:], octs[-1][:], OP.add)
                                dps_ps = spool.tile([128, 1024], F32,
                                                    tag="dps", bufs=1,
                                                    name="dpsum")
                                for qc2 in range(2):
                                    mm_hold[0] = nc.tensor.matmul(
                                        dps_ps[:, qc2 * 512:(qc2 + 1) * 512],
                                        ones_sb[:],
                                        e16[:, qc2 * 512:(qc2 + 1) * 512],
                                        start=True, stop=True)
                                if last_block:
                                    # free the ops psum early: the norm reads
                                    # an SBUF copy, so the oproj psum pool's
                                    # boundary only waits on the dps recs
                                    opc = [onpool.tile([128, 512], F32,
                                                       tag="opscopy", bufs=2,
                                                       name="opsc")
                                           for _ in range(2)]
                                    for qc2 in range(2):
                                        nc.vector.tensor_copy(
                                            opc[qc2][:], ops_ps[qc2][:])
                                    emit_norm_a2a(
                                        h, b, qp,
                                        lambda qc2: dps_ps[:, qc2 * 512:
                                                           (qc2 + 1) * 512],
                                        lambda qc2: opc[qc2][:])
                                else:
                                    emit_norm_a2a(
                                        h, b, qp,
                                        lambda qc2: dps_ps[:, qc2 * 512:
                                                           (qc2 + 1) * 512],
                                        lambda qc2: ops_ps[qc2][:])
                        emit_collective_fetch(h)
                        if h == 0:
                            # wo is only needed at the output projection; fetch it
                            # during SDPA when HBM is otherwise quiet
                            nc.scalar.dma_start(wo_sb[:], wo_t[:])

                # ---------- phase D: output projection, split per head ----------
                # head-0's half runs while head-1's AllToAll is in flight
                with (
                    tc.tile_pool(name="ysb", bufs=1) as ypool,
                    tc.tile_pool(name="ysum", bufs=4) as ysumpool,
                    tc.tile_pool(name="yps", bufs=2, space="PSUM") as yppool,
                ):
                    # ordering anchor: keep the output-projection chains after
                    # the SDPA tail in the PE stream — the scheduler's cost
                    # model undershoots the AllToAll latency and would
                    # otherwise front-run these and stall the PE
                    anchor = mm_hold[0]
                    for hh in range(HL):
                        if hh == 1:
                            # bridge a possible wait for the second AllToAll
                            # with a few throwaway matmuls (the first real
                            # start=True clears the bank). Anchor them on the
                            # last h0 chain's FINAL matmul so they cannot
                            # interleave into that chain.
                            anchor = last_mm
                            warm_yp = yppool.tile([128, 512], F32, tag="yps",
                                                  name="warmyp")
                            for _ in range(12):
                                wmm = nc.tensor.matmul(
                                    warm_yp[:, 0:128], ones_sb[:], ones_sb[:],
                                    start=True, stop=True)
                                bass_rust.add_dep_helper(
                                    wmm.ins, anchor.ins, sync=False,
                                    reason="keep PE warm across a2a#1 wait")
                                anchor = wmm
                        for tt in range(TL // 128):
                            for eo in range(4):
                                yp = yppool.tile([128, 512], F32, tag="yps",
                                                 name="ypsum")
                                for di in range(NCORES):
                                    d = di * HL + hh
                                    mm = nc.tensor.matmul(
                                        yp[:],
                                        ot_sb[hh][:, di * TL + tt * 128:
                                                  di * TL + (tt + 1) * 128],
                                        wo_sb[:, d * D + eo * 512:
                                              d * D + (eo + 1) * 512],
                                        start=(di == 0), stop=(di == NCORES - 1))
                                    if di == 0:
                                        bass_rust.add_dep_helper(
                                            mm.ins, anchor.ins, sync=False,
                                            reason="order oproj after prior phase")
                                        anchor = mm
                                    last_mm = mm
                                if hh == 0:
                                    y0 = ypool.tile([128, 512], F32,
                                                    tag=f"y0_{tt}_{eo}",
                                                    name=f"y0_{tt}_{eo}")
                                    nc.vector.tensor_copy(y0[:], yp[:])
                                    ot_sb[(0, tt, eo)] = y0
                                else:
                                    ys = ysumpool.tile([128, 512], F32, tag="ysum",
                                                       name="ysum")
                                    last_chunk = (tt == TL // 128 - 1
                                                  and eo == 3)
                                    if not last_chunk:
                                        nc.vector.tensor_tensor(
                                            ys[:], yp[:],
                                            ot_sb[(0, tt, eo)][:], OP.add)
                                        nc.sync.dma_start(
                                            out[tt * 128:(tt + 1) * 128,
                                                eo * 512:(eo + 1) * 512], ys[:])
                                    else:
                                        # split the very last add+store so the
                                        # end-of-kernel drain waits on a 128KB
                                        # DMA, not a 256KB one
                                        for ch in range(2):
                                            cs = slice(ch * 256, (ch + 1) * 256)
                                            nc.vector.tensor_tensor(
                                                ys[:, cs], yp[:, cs],
                                                ot_sb[(0, tt, eo)][:, cs],
                                                OP.add)
                                            nc.sync.dma_start(
                                                out[tt * 128:(tt + 1) * 128,
                                                    eo * 512 + ch * 256:
                                                    eo * 512 + (ch + 1) * 256],
                                                ys[:, cs])


    nc.compile()
    return nc


def _prep_in_maps(x, cos, sin, attn_mask, wq, wk, wv, wo):
    x_t = np.ascontiguousarray(
        np.asarray(x, np.float32).reshape(T, D).T.astype(bf16))      # [D, T]
    cosT = np.ascontiguousarray(np.asarray(cos[0], np.float32).T)    # [HD, S]
    sinT = np.asarray(sin[0], np.float32).T
    sin_m = np.ascontiguousarray(
        np.concatenate([-sinT[:64], sinT[64:]], axis=0))             # [HD, S]
    mask_t = np.ascontiguousarray(
        np.asarray(attn_mask, np.float32).reshape(B * NKT, 128).T)   # [128, 32]

    def pack(w_sl):
        # [E_out, D] slice -> [128, NDT * E_out] d-tile-major layout
        e_out = w_sl.shape[0]
        return np.ascontiguousarray(
            w_sl.T.reshape(NDT, 128, e_out).transpose(1, 0, 2)
            .reshape(128, NDT * e_out).astype(bf16))

    wo_t = pack(np.asarray(wo, np.float32))
    in_maps = []
    for i in range(NCORES):
        sl = slice(i * EL, (i + 1) * EL)
        in_maps.append({
            "x_t": x_t,
            "wq_t": pack(np.asarray(wq, np.float32)[sl]),
            "wk_t": pack(np.asarray(wk, np.float32)[sl]),
            "wv_t": pack(np.asarray(wv, np.float32)[sl]),
            "wo_t": wo_t,
            "cos_t": cosT.astype(bf16),
            "sin_m": sin_m.astype(bf16),
            "mask_t": mask_t,
        })
    return in_maps


def kernel(x, cos, sin, attn_mask, wq, wk, wv, wo, _trace=False):
    if "nc" not in _CACHE:
        _CACHE["nc"] = _build()
    nc = _CACHE["nc"]
    in_maps = _prep_in_maps(x, cos, sin, attn_mask, wq, wk, wv, wo)
    res = run_bass_kernel_spmd(nc, in_maps, core_ids=list(range(NCORES)),
                               trace=_trace)
    _CACHE["last_result"] = res
    y = np.concatenate([np.asarray(res.results[i]["out"], np.float32)
                        for i in range(NCORES)], axis=0)
    return y.reshape(B, S, D)

